# revision 17
# baseline (speedup 1.0000x reference)
"""DoRA linear layer on 8 TRN2 NeuronCores (bf16 tensor-engine path).

out = (magnitude / ||W + s*B@A||_row) * (x @ (W + s*B@A)^T),  s = alpha/rank = 2.

Identity used: the reference's
    dora_out + base_out = mag_norm_scale * (base_out + s * lora_out)
                        = scale_o * (x @ W^T + s * (x @ A^T) @ B^T)

Structure (per core, data-parallel over tokens, 1024 tok/core):
  - stationary = W^T chunk [128i, 128o], moving = x^T [128i, 512t] (bf16:
    1 col/cycle vs ~2 for fp32r on real HW) -> psum out^T tiles [128o, 512t].
  - n2 = 2*(W @ A^T) rides the same stationary as extra 16-col matmuls into
    a [128o, 16] psum; B2@G accumulates into the same psum, so the row norm
    finishes as ONE fused multiply-accumulate against B2 (natural layout)
    plus n1 = rowsum(W^2) from an fp16 W copy on the scalar engine.
  - All norm/scale math lives in o-partition space: scale is a [128,1]
    per-partition broadcast, no transposes, no DRAM round-trip.
  - out^T written bf16; host transposes/casts back to [tok, out] fp32.
"""

import sys

sys.path.insert(0, "/opt/trn_rl_repo")

import numpy as np
import ml_dtypes

import concourse.bass as bass  # noqa: F401  (import keeps bass registered)
from concourse import bacc
import concourse.mybir as mybir
from concourse.tile import TileContext
from concourse.bass_utils import run_bass_kernel_spmd

FP32 = mybir.dt.float32
BF16 = mybir.dt.bfloat16
FP16 = mybir.dt.float16
FP8 = mybir.dt.float8e4

BF = ml_dtypes.bfloat16

NCORES = 8
TOK = 8192          # 4 * 2048 tokens
TPC = TOK // NCORES  # 1024 tokens per core
DIN = 4096
DOUT = 4096
RANK = 16
SCALING = 32.0 / 16

NI = DIN // 128      # 32 contraction blocks
NOC = DOUT // 128    # 32 output chunks of 128
NXQ = 4              # x DMA split (ib-groups)


def _build_program():
    nc = bacc.Bacc("TRN2", target_bir_lowering=False, debug=False,
                   num_devices=NCORES)

    xt_d = nc.dram_tensor("xt", [128, NI, TPC], BF16, kind="ExternalInput")
    wt_d = nc.dram_tensor("wt", [NOC, 128, NI, 128], BF16, kind="ExternalInput")
    wh_d = nc.dram_tensor("wh", [128, NOC, DIN], FP8, kind="ExternalInput")
    at_d = nc.dram_tensor("at", [128, NI, RANK], BF16, kind="ExternalInput")
    a2t_d = nc.dram_tensor("a2t", [128, NI, RANK], BF16, kind="ExternalInput")
    b2t_d = nc.dram_tensor("b2t", [RANK, DOUT], BF16, kind="ExternalInput")
    b2n_d = nc.dram_tensor("b2n", [128, NOC, RANK], BF16, kind="ExternalInput")
    mag_d = nc.dram_tensor("mag", [128, NOC], FP32, kind="ExternalInput")
    out_d = nc.dram_tensor("out", [DOUT, TPC], BF16, kind="ExternalOutput")

    with TileContext(nc) as tc:
        with (
            tc.tile_pool(name="const", bufs=1) as const,
            tc.tile_pool(name="xtp", bufs=1) as xtp,
            tc.tile_pool(name="wtp", bufs=4) as wtp,
            tc.tile_pool(name="whp", bufs=3) as whp,
            tc.tile_pool(name="outp", bufs=6) as outp,
            tc.tile_pool(name="scl", bufs=4) as scl,
            tc.tile_pool(name="mp", bufs=4, space="PSUM") as mp,
            tc.tile_pool(name="np2", bufs=2, space="PSUM") as np2,
            tc.tile_pool(name="xap", bufs=2, space="PSUM") as xap,
        ):
            aT = const.tile([128, NI, RANK], BF16)
            nc.sync.dma_start(aT[:], at_d[:])
            a2T = const.tile([128, NI, RANK], BF16)
            nc.sync.dma_start(a2T[:], a2t_d[:])

            # resident x^T [i_part, i_blk, tok]: one DMA per i-block — a
            # single DMA queue moves only ~23 GB/s, so landing time is set
            # by how many queues a tensor is spread across
            QI = NI // NXQ
            xTq = [xtp.tile([128, QI, TPC], BF16, name=f"xTq{q}")
                   for q in range(NXQ)]
            b2t = const.tile([RANK, DOUT], BF16)
            b2n = const.tile([128, NOC, RANK], BF16)
            mag = const.tile([128, NOC], FP32)

            def xT(ib):
                return xTq[ib // QI][:, ib % QI, :]

            def dma_xq(q):
                for j in range(QI):
                    for h in range(2):
                        nc.sync.dma_start(
                            xTq[q][:, j, h * 512:(h + 1) * 512],
                            xt_d[:, q * QI + j, h * 512:(h + 1) * 512])

            dma_xq(0)

            def chunk_tiles(c, with_wh=True):
                wt_c = wtp.tile([128, NI, 128], BF16, tag="w", name=f"w{c}")
                for h in range(8):
                    nc.sync.dma_start(wt_c[:, h * 4:(h + 1) * 4, :],
                                      wt_d[c, :, h * 4:(h + 1) * 4, :])
                t = {"c": c, "wt": wt_c}
                if with_wh:
                    chunk_wh(t)
                return t

            def chunk_wh(t):
                c = t["c"]
                # n1 = rowsum(W^2) on the scalar engine (fp16 natural
                # layout); one DMA piece per ACT op
                wh_c = whp.tile([128, DIN], FP8, tag="wh", name=f"wh{c}")
                n1p = scl.tile([128, 4], FP32, tag="n1p", name=f"n1p{c}")
                for k in range(4):
                    nc.sync.dma_start(wh_c[:, k * 1024:(k + 1) * 1024],
                                      wh_d[:, c, k * 1024:(k + 1) * 1024])
                    nc.scalar.activation(
                        scl.tile([128, 1024], FP32, tag="sqw",
                                 name=f"sqw{c}_{k}")[:],
                        wh_c[:, k * 1024:(k + 1) * 1024],
                        mybir.ActivationFunctionType.Square,
                        accum_out=n1p[:, k:k + 1])
                n1c = scl.tile([128, 1], FP32, tag="n1c", name=f"n1c{c}")
                nc.vector.reduce_sum(n1c[:], n1p[:], axis=mybir.AxisListType.X)
                t["n1c"] = n1c

            def chunk_psums(t):
                c = t["c"]
                t["ps0"] = mp.tile([128, 512], FP32, tag="mp", name=f"ps0_{c}")
                t["ps1"] = mp.tile([128, 512], FP32, tag="mp", name=f"ps1_{c}")
                t["pn2"] = np2.tile([128, RANK], FP32, tag="np2",
                                    name=f"pn2_{c}")

            def chunk_ib(t, ib, with_pn2=True):
                w = t["wt"][:, ib, :]
                nc.tensor.matmul(t["ps0"][:], w, xT(ib)[:, 0:512],
                                 start=(ib == 0), stop=False)
                if with_pn2:
                    nc.tensor.matmul(t["pn2"][:], w, a2T[:, ib, :],
                                     start=(ib == 0), stop=False)
                nc.tensor.matmul(t["ps1"][:], w, xT(ib)[:, 512:1024],
                                 start=(ib == 0), stop=False)

            def pn2_pass(t):
                for ib in range(NI):
                    nc.tensor.matmul(t["pn2"][:], t["wt"][:, ib, :],
                                     a2T[:, ib, :],
                                     start=(ib == 0), stop=False)

            def finish_norm(t):
                c = t["c"]
                pn2 = t["pn2"]
                b2c = b2t[:, c * 128:(c + 1) * 128]
                # + B2@G into the n2 psum: row norm finishes as one fused
                # multiply-accumulate against B2
                nc.tensor.matmul(pn2[:], b2c, g_sb[:], start=False, stop=True)

                # cross + lowrank norm terms: sum_r pn2[o,r] * B2[o,r]
                cr = scl.tile([128, 1], FP32, tag="cr", name=f"cr{c}")
                nc.vector.scalar_tensor_tensor(
                    out=scl.tile([128, RANK], FP32, tag="scr",
                                 name=f"scr{c}")[:],
                    in0=pn2[:], scalar=1.0, in1=b2n[:, c, :],
                    op0=mybir.AluOpType.mult, op1=mybir.AluOpType.mult,
                    accum_out=cr[:])
                nsq = scl.tile([128, 1], FP32, tag="nsq", name=f"nsq{c}")
                nc.vector.tensor_add(nsq[:], cr[:], t["n1c"][:])
                nrm = scl.tile([128, 1], FP32, tag="nrm", name=f"nrm{c}")
                nc.scalar.activation(nrm[:], nsq[:],
                                     mybir.ActivationFunctionType.Sqrt)
                nc.vector.reciprocal(nrm[:], nrm[:])
                sc = scl.tile([128, 1], FP32, tag="sc", name=f"sc{c}")
                nc.vector.tensor_mul(sc[:], nrm[:], mag[:, c:c + 1])
                t["sc"] = sc

            def finish_apply(t, out_pieces=1):
                c = t["c"]
                b2c = b2t[:, c * 128:(c + 1) * 128]
                # rank-16 DoRA term folded into the out accumulation
                nc.tensor.matmul(t["ps0"][:], b2c, xaT[:, 0:512],
                                 start=False, stop=True)
                nc.tensor.matmul(t["ps1"][:], b2c, xaT[:, 512:1024],
                                 start=False, stop=True)
                for half, ps in ((0, t["ps0"]), (1, t["ps1"])):
                    o_t = outp.tile([128, 512], BF16, tag="o",
                                    name=f"o{half}_{c}")
                    nc.vector.tensor_scalar_mul(o_t[:], ps[:], t["sc"][:])
                    w_p = 512 // out_pieces
                    for h in range(out_pieces):
                        nc.sync.dma_start(
                            out_d[c * 128:(c + 1) * 128,
                                  half * 512 + h * w_p:
                                  half * 512 + (h + 1) * w_p],
                            o_t[:, h * w_p:(h + 1) * w_p])

            # ---- chunks 0+1, interleaved per x i-block group to race the
            # x DMA; xa accumulates alongside ----
            t0 = chunk_tiles(0, with_wh=False)
            t1 = chunk_tiles(1, with_wh=False)
            chunk_psums(t0)
            chunk_psums(t1)

            # G = A @ A^T  [rank, rank] — needs only aT, runs immediately
            # (borrows an np2 bank; drained well before pn2_1 needs it)
            ps_g = np2.tile([RANK, RANK], FP32, tag="np2", name="psg")
            for ib in range(NI):
                nc.tensor.matmul(ps_g[:], aT[:, ib, :], aT[:, ib, :],
                                 start=(ib == 0), stop=(ib == NI - 1))
            g_sb = const.tile([RANK, RANK], BF16)
            nc.vector.tensor_copy(g_sb[:], ps_g[:])

            # xa^T = (x @ A^T)^T [rank, tok]
            xaT = const.tile([RANK, TPC], BF16)
            ps_xa = [xap.tile([RANK, 512], FP32, tag="xap", name=f"psxa{q}")
                     for q in range(2)]

            for q in range(NXQ):
                if q + 1 < NXQ:
                    dma_xq(q + 1)
                if q == 1:
                    chunk_wh(t0)
                    nc.sync.dma_start(b2t[:], b2t_d[:])
                if q == 2:
                    chunk_wh(t1)
                    nc.sync.dma_start(b2n[:], b2n_d[:])
                    nc.sync.dma_start(mag[:], mag_d[:])
                for j in range(QI):
                    ib = q * QI + j
                    nc.tensor.matmul(ps_xa[0][:], aT[:, ib, :],
                                     xT(ib)[:, 0:512],
                                     start=(ib == 0), stop=(ib == NI - 1))
                    nc.tensor.matmul(ps_xa[1][:], aT[:, ib, :],
                                     xT(ib)[:, 512:1024],
                                     start=(ib == 0), stop=(ib == NI - 1))
                    chunk_ib(t0, ib)
                    chunk_ib(t1, ib)
            for q in range(2):
                nc.vector.tensor_copy(xaT[:, q * 512:(q + 1) * 512],
                                      ps_xa[q][:])
            finish_norm(t0)
            finish_apply(t0)
            finish_norm(t1)
            finish_apply(t1)

            # ---- steady-state chunks: pn2 + norm chain first so the last
            # chunk's norm math hides under its own main matmuls ----
            for c in range(2, NOC):
                t = chunk_tiles(c)
                chunk_psums(t)
                pn2_pass(t)
                finish_norm(t)
                for ib in range(NI):
                    chunk_ib(t, ib, with_pn2=False)
                finish_apply(t, out_pieces=4 if c == NOC - 1 else 1)

    nc.compile()
    return nc


_PROGRAM = None


def _get_program():
    global _PROGRAM
    if _PROGRAM is None:
        _PROGRAM = _build_program()
    return _PROGRAM


def _prep_inputs(x, weight, lora_a_w, lora_b_w, magnitude):
    w32 = weight.astype(np.float32)
    wb = w32.astype(BF)
    wt = np.ascontiguousarray(
        wb.T.reshape(NI, 128, NOC, 128).transpose(2, 1, 0, 3))
    wh = np.ascontiguousarray(
        w32.astype(ml_dtypes.float8_e4m3).reshape(NOC, 128, DIN).transpose(1, 0, 2))
    a32 = lora_a_w.astype(np.float32)
    at = np.ascontiguousarray(
        a32.astype(BF).T.reshape(NI, 128, RANK).transpose(1, 0, 2))
    a2t = np.ascontiguousarray(
        (2.0 * a32).astype(BF).T.reshape(NI, 128, RANK).transpose(1, 0, 2))
    b2 = (SCALING * lora_b_w.astype(np.float32)).astype(BF)
    b2t = np.ascontiguousarray(b2.T)
    b2n = np.ascontiguousarray(
        b2.reshape(NOC, 128, RANK).transpose(1, 0, 2))
    magr = np.ascontiguousarray(
        magnitude.astype(np.float32).reshape(NOC, 128).T)

    xb = x.reshape(TOK, DIN).astype(BF)
    in_maps = []
    for cpu in range(NCORES):
        xs = xb[cpu * TPC:(cpu + 1) * TPC].T
        xt = np.ascontiguousarray(
            xs.reshape(NI, 128, TPC).transpose(1, 0, 2))
        in_maps.append({"xt": xt, "wt": wt, "wh": wh, "at": at, "a2t": a2t,
                        "b2t": b2t, "b2n": b2n, "mag": magr})
    return in_maps


def kernel(x, weight, lora_a_w, lora_b_w, magnitude, _trace=False, **_kw):
    nc = _get_program()
    in_maps = _prep_inputs(x, weight, lora_a_w, lora_b_w, magnitude)
    res = run_bass_kernel_spmd(nc, in_maps, list(range(NCORES)), trace=_trace)
    out = np.empty((TOK, DOUT), dtype=np.float32)
    for c in range(NCORES):
        out[c * TPC:(c + 1) * TPC] = res.results[c]["out"].T.astype(np.float32)
    if _trace:
        kernel._last_results = res
    return out.reshape(4, 2048, DOUT)


# revision 18
# speedup vs baseline: 1.0088x; 1.0088x over previous
"""DoRA linear layer on 8 TRN2 NeuronCores (bf16 tensor-engine path).

out = (magnitude / ||W + s*B@A||_row) * (x @ (W + s*B@A)^T),  s = alpha/rank = 2.

Identity used: the reference's
    dora_out + base_out = mag_norm_scale * (base_out + s * lora_out)
                        = scale_o * (x @ W^T + s * (x @ A^T) @ B^T)

Structure (per core, data-parallel over tokens, 1024 tok/core):
  - stationary = W^T chunk [128i, 128o], moving = x^T [128i, 512t] (bf16:
    1 col/cycle vs ~2 for fp32r on real HW) -> psum out^T tiles [128o, 512t].
  - n2 = 2*(W @ A^T) rides the same stationary as extra 16-col matmuls into
    a [128o, 16] psum; B2@G accumulates into the same psum, so the row norm
    finishes as ONE fused multiply-accumulate against B2 (natural layout)
    plus n1 = rowsum(W^2) from an fp16 W copy on the scalar engine.
  - All norm/scale math lives in o-partition space: scale is a [128,1]
    per-partition broadcast, no transposes, no DRAM round-trip.
  - out^T written bf16; host transposes/casts back to [tok, out] fp32.
"""

import sys

sys.path.insert(0, "/opt/trn_rl_repo")

import numpy as np
import ml_dtypes

import concourse.bass as bass  # noqa: F401  (import keeps bass registered)
from concourse import bacc
import concourse.mybir as mybir
from concourse.tile import TileContext
from concourse.bass_utils import run_bass_kernel_spmd

FP32 = mybir.dt.float32
BF16 = mybir.dt.bfloat16
FP16 = mybir.dt.float16
FP8 = mybir.dt.float8e4

BF = ml_dtypes.bfloat16

NCORES = 8
TOK = 8192          # 4 * 2048 tokens
TPC = TOK // NCORES  # 1024 tokens per core
DIN = 4096
DOUT = 4096
RANK = 16
SCALING = 32.0 / 16

NI = DIN // 128      # 32 contraction blocks
NOC = DOUT // 128    # 32 output chunks of 128
NXQ = 4              # x DMA split (ib-groups)


def _build_program():
    nc = bacc.Bacc("TRN2", target_bir_lowering=False, debug=False,
                   num_devices=NCORES)

    xt_d = nc.dram_tensor("xt", [128, NI, TPC], BF16, kind="ExternalInput")
    wt_d = nc.dram_tensor("wt", [NOC, 128, NI, 128], BF16, kind="ExternalInput")
    wh_d = nc.dram_tensor("wh", [128, NOC, DIN], FP8, kind="ExternalInput")
    at_d = nc.dram_tensor("at", [128, NI, RANK], BF16, kind="ExternalInput")
    a2t_d = nc.dram_tensor("a2t", [128, NI, RANK], BF16, kind="ExternalInput")
    b2t_d = nc.dram_tensor("b2t", [RANK, DOUT], BF16, kind="ExternalInput")
    b2n_d = nc.dram_tensor("b2n", [128, NOC, RANK], BF16, kind="ExternalInput")
    mag_d = nc.dram_tensor("mag", [128, NOC], FP32, kind="ExternalInput")
    out_d = nc.dram_tensor("out", [DOUT, TPC], BF16, kind="ExternalOutput")

    with TileContext(nc) as tc:
        with (
            tc.tile_pool(name="const", bufs=1) as const,
            tc.tile_pool(name="xtp", bufs=1) as xtp,
            tc.tile_pool(name="wtp", bufs=4) as wtp,
            tc.tile_pool(name="whp", bufs=3) as whp,
            tc.tile_pool(name="outp", bufs=6) as outp,
            tc.tile_pool(name="scl", bufs=4) as scl,
            tc.tile_pool(name="mp", bufs=4, space="PSUM") as mp,
            tc.tile_pool(name="np2", bufs=2, space="PSUM") as np2,
            tc.tile_pool(name="xap", bufs=2, space="PSUM") as xap,
        ):
            aT = const.tile([128, NI, RANK], BF16)
            nc.sync.dma_start(aT[:], at_d[:])
            a2T = const.tile([128, NI, RANK], BF16)
            nc.sync.dma_start(a2T[:], a2t_d[:])

            # resident x^T [i_part, i_blk, tok]: one DMA per i-block — a
            # single DMA queue moves only ~23 GB/s, so landing time is set
            # by how many queues a tensor is spread across
            QI = NI // NXQ
            xTq = [xtp.tile([128, QI, TPC], BF16, name=f"xTq{q}")
                   for q in range(NXQ)]
            b2t = const.tile([RANK, DOUT], BF16)
            b2n = const.tile([128, NOC, RANK], BF16)
            mag = const.tile([128, NOC], FP32)

            def xT(ib):
                return xTq[ib // QI][:, ib % QI, :]

            def dma_xq(q):
                for j in range(QI):
                    for h in range(2):
                        nc.sync.dma_start(
                            xTq[q][:, j, h * 512:(h + 1) * 512],
                            xt_d[:, q * QI + j, h * 512:(h + 1) * 512])

            dma_xq(0)

            def chunk_tiles(c, with_wh=True):
                wt_c = wtp.tile([128, NI, 128], BF16, tag="w", name=f"w{c}")
                for h in range(8):
                    nc.sync.dma_start(wt_c[:, h * 4:(h + 1) * 4, :],
                                      wt_d[c, :, h * 4:(h + 1) * 4, :])
                t = {"c": c, "wt": wt_c}
                if with_wh:
                    chunk_wh(t)
                return t

            def chunk_wh(t):
                c = t["c"]
                # n1 = rowsum(W^2) on the scalar engine (fp16 natural
                # layout); one DMA piece per ACT op
                wh_c = whp.tile([128, DIN], FP8, tag="wh", name=f"wh{c}")
                n1p = scl.tile([128, 4], FP32, tag="n1p", name=f"n1p{c}")
                for k in range(4):
                    nc.sync.dma_start(wh_c[:, k * 1024:(k + 1) * 1024],
                                      wh_d[:, c, k * 1024:(k + 1) * 1024])
                    nc.scalar.activation(
                        scl.tile([128, 1024], FP32, tag="sqw",
                                 name=f"sqw{c}_{k}")[:],
                        wh_c[:, k * 1024:(k + 1) * 1024],
                        mybir.ActivationFunctionType.Square,
                        accum_out=n1p[:, k:k + 1])
                n1c = scl.tile([128, 1], FP32, tag="n1c", name=f"n1c{c}")
                nc.vector.reduce_sum(n1c[:], n1p[:], axis=mybir.AxisListType.X)
                t["n1c"] = n1c

            def chunk_psums(t):
                c = t["c"]
                t["ps0"] = mp.tile([128, 512], FP32, tag="mp", name=f"ps0_{c}")
                t["ps1"] = mp.tile([128, 512], FP32, tag="mp", name=f"ps1_{c}")
                t["pn2"] = np2.tile([128, RANK], FP32, tag="np2",
                                    name=f"pn2_{c}")

            def chunk_ib(t, ib, with_pn2=True):
                w = t["wt"][:, ib, :]
                nc.tensor.matmul(t["ps0"][:], w, xT(ib)[:, 0:512],
                                 start=(ib == 0), stop=False)
                if with_pn2:
                    nc.tensor.matmul(t["pn2"][:], w, a2T[:, ib, :],
                                     start=(ib == 0), stop=False)
                nc.tensor.matmul(t["ps1"][:], w, xT(ib)[:, 512:1024],
                                 start=(ib == 0), stop=False)

            def pn2_pass(t):
                for ib in range(NI):
                    nc.tensor.matmul(t["pn2"][:], t["wt"][:, ib, :],
                                     a2T[:, ib, :],
                                     start=(ib == 0), stop=False)

            def finish_norm(t):
                c = t["c"]
                pn2 = t["pn2"]
                b2c = b2t[:, c * 128:(c + 1) * 128]
                # + B2@G into the n2 psum: row norm finishes as one fused
                # multiply-accumulate against B2
                nc.tensor.matmul(pn2[:], b2c, g_sb[:], start=False, stop=True)

                # cross + lowrank norm terms: sum_r pn2[o,r] * B2[o,r]
                cr = scl.tile([128, 1], FP32, tag="cr", name=f"cr{c}")
                nc.vector.scalar_tensor_tensor(
                    out=scl.tile([128, RANK], FP32, tag="scr",
                                 name=f"scr{c}")[:],
                    in0=pn2[:], scalar=1.0, in1=b2n[:, c, :],
                    op0=mybir.AluOpType.mult, op1=mybir.AluOpType.mult,
                    accum_out=cr[:])
                nsq = scl.tile([128, 1], FP32, tag="nsq", name=f"nsq{c}")
                nc.vector.tensor_add(nsq[:], cr[:], t["n1c"][:])
                nrm = scl.tile([128, 1], FP32, tag="nrm", name=f"nrm{c}")
                nc.scalar.activation(nrm[:], nsq[:],
                                     mybir.ActivationFunctionType.Sqrt)
                nc.vector.reciprocal(nrm[:], nrm[:])
                sc = scl.tile([128, 1], FP32, tag="sc", name=f"sc{c}")
                nc.vector.tensor_mul(sc[:], nrm[:], mag[:, c:c + 1])
                t["sc"] = sc

            def finish_apply(t, out_pieces=1):
                c = t["c"]
                b2c = b2t[:, c * 128:(c + 1) * 128]
                # rank-16 DoRA term folded into the out accumulation
                nc.tensor.matmul(t["ps0"][:], b2c, xaT[:, 0:512],
                                 start=False, stop=True)
                nc.tensor.matmul(t["ps1"][:], b2c, xaT[:, 512:1024],
                                 start=False, stop=True)
                for half, ps in ((0, t["ps0"]), (1, t["ps1"])):
                    o_t = outp.tile([128, 512], BF16, tag="o",
                                    name=f"o{half}_{c}")
                    nc.vector.tensor_scalar_mul(o_t[:], ps[:], t["sc"][:])
                    w_p = 512 // out_pieces
                    for h in range(out_pieces):
                        nc.sync.dma_start(
                            out_d[c * 128:(c + 1) * 128,
                                  half * 512 + h * w_p:
                                  half * 512 + (h + 1) * w_p],
                            o_t[:, h * w_p:(h + 1) * w_p])

            # ---- chunks 0+1, interleaved per x i-block group to race the
            # x DMA; xa accumulates alongside ----
            t0 = chunk_tiles(0, with_wh=False)
            t1 = chunk_tiles(1, with_wh=False)
            chunk_psums(t0)
            chunk_psums(t1)

            # G = A @ A^T  [rank, rank] — needs only aT, runs immediately
            # (borrows an np2 bank; drained well before pn2_1 needs it)
            ps_g = np2.tile([RANK, RANK], FP32, tag="np2", name="psg")
            for ib in range(NI):
                nc.tensor.matmul(ps_g[:], aT[:, ib, :], aT[:, ib, :],
                                 start=(ib == 0), stop=(ib == NI - 1))
            g_sb = const.tile([RANK, RANK], BF16)
            nc.vector.tensor_copy(g_sb[:], ps_g[:])

            # xa^T = (x @ A^T)^T [rank, tok]
            xaT = const.tile([RANK, TPC], BF16)
            ps_xa = [xap.tile([RANK, 512], FP32, tag="xap", name=f"psxa{q}")
                     for q in range(2)]

            for q in range(NXQ):
                if q + 1 < NXQ:
                    dma_xq(q + 1)
                if q == 1:
                    chunk_wh(t0)
                    nc.sync.dma_start(b2t[:], b2t_d[:])
                if q == 2:
                    chunk_wh(t1)
                    nc.sync.dma_start(b2n[:], b2n_d[:])
                    nc.sync.dma_start(mag[:], mag_d[:])
                for j in range(QI):
                    ib = q * QI + j
                    nc.tensor.matmul(ps_xa[0][:], aT[:, ib, :],
                                     xT(ib)[:, 0:512],
                                     start=(ib == 0), stop=(ib == NI - 1))
                    nc.tensor.matmul(ps_xa[1][:], aT[:, ib, :],
                                     xT(ib)[:, 512:1024],
                                     start=(ib == 0), stop=(ib == NI - 1))
                    chunk_ib(t0, ib)
                    chunk_ib(t1, ib)
            for q in range(2):
                nc.vector.tensor_copy(xaT[:, q * 512:(q + 1) * 512],
                                      ps_xa[q][:])
            finish_norm(t0)
            finish_apply(t0)
            finish_norm(t1)
            finish_apply(t1)

            # ---- steady-state chunks ----
            for c in range(2, NOC - 1):
                t = chunk_tiles(c)
                chunk_psums(t)
                for ib in range(NI):
                    chunk_ib(t, ib)
                finish_norm(t)
                finish_apply(t)

            # last chunk: its wt is fully prefetched, so run the pn2 pass +
            # norm chain first — the tail then ends right after the folds
            t = chunk_tiles(NOC - 1)
            chunk_psums(t)
            pn2_pass(t)
            finish_norm(t)
            for ib in range(NI):
                chunk_ib(t, ib, with_pn2=False)
            finish_apply(t, out_pieces=4)

    nc.compile()
    return nc


_PROGRAM = None


def _get_program():
    global _PROGRAM
    if _PROGRAM is None:
        _PROGRAM = _build_program()
    return _PROGRAM


def _prep_inputs(x, weight, lora_a_w, lora_b_w, magnitude):
    w32 = weight.astype(np.float32)
    wb = w32.astype(BF)
    wt = np.ascontiguousarray(
        wb.T.reshape(NI, 128, NOC, 128).transpose(2, 1, 0, 3))
    wh = np.ascontiguousarray(
        w32.astype(ml_dtypes.float8_e4m3).reshape(NOC, 128, DIN).transpose(1, 0, 2))
    a32 = lora_a_w.astype(np.float32)
    at = np.ascontiguousarray(
        a32.astype(BF).T.reshape(NI, 128, RANK).transpose(1, 0, 2))
    a2t = np.ascontiguousarray(
        (2.0 * a32).astype(BF).T.reshape(NI, 128, RANK).transpose(1, 0, 2))
    b2 = (SCALING * lora_b_w.astype(np.float32)).astype(BF)
    b2t = np.ascontiguousarray(b2.T)
    b2n = np.ascontiguousarray(
        b2.reshape(NOC, 128, RANK).transpose(1, 0, 2))
    magr = np.ascontiguousarray(
        magnitude.astype(np.float32).reshape(NOC, 128).T)

    xb = x.reshape(TOK, DIN).astype(BF)
    in_maps = []
    for cpu in range(NCORES):
        xs = xb[cpu * TPC:(cpu + 1) * TPC].T
        xt = np.ascontiguousarray(
            xs.reshape(NI, 128, TPC).transpose(1, 0, 2))
        in_maps.append({"xt": xt, "wt": wt, "wh": wh, "at": at, "a2t": a2t,
                        "b2t": b2t, "b2n": b2n, "mag": magr})
    return in_maps


def kernel(x, weight, lora_a_w, lora_b_w, magnitude, _trace=False, **_kw):
    nc = _get_program()
    in_maps = _prep_inputs(x, weight, lora_a_w, lora_b_w, magnitude)
    res = run_bass_kernel_spmd(nc, in_maps, list(range(NCORES)), trace=_trace)
    out = np.empty((TOK, DOUT), dtype=np.float32)
    for c in range(NCORES):
        out[c * TPC:(c + 1) * TPC] = res.results[c]["out"].T.astype(np.float32)
    if _trace:
        kernel._last_results = res
    return out.reshape(4, 2048, DOUT)


# revision 19
# speedup vs baseline: 1.0300x; 1.0210x over previous
"""DoRA linear layer on 8 TRN2 NeuronCores (bf16 tensor-engine path).

out = (magnitude / ||W + s*B@A||_row) * (x @ (W + s*B@A)^T),  s = alpha/rank = 2.

Identity used: the reference's
    dora_out + base_out = mag_norm_scale * (base_out + s * lora_out)
                        = scale_o * (x @ W^T + s * (x @ A^T) @ B^T)

Structure (per core, data-parallel over tokens, 1024 tok/core):
  - stationary = W^T chunk [128i, 128o], moving = x^T [128i, 512t] (bf16:
    1 col/cycle vs ~2 for fp32r on real HW) -> psum out^T tiles [128o, 512t].
  - n2 = 2*(W @ A^T) rides the same stationary as extra 16-col matmuls into
    a [128o, 16] psum; B2@G accumulates into the same psum, so the row norm
    finishes as ONE fused multiply-accumulate against B2 (natural layout)
    plus n1 = rowsum(W^2) from an fp16 W copy on the scalar engine.
  - All norm/scale math lives in o-partition space: scale is a [128,1]
    per-partition broadcast, no transposes, no DRAM round-trip.
  - out^T written bf16; host transposes/casts back to [tok, out] fp32.
"""

import sys

sys.path.insert(0, "/opt/trn_rl_repo")

import numpy as np
import ml_dtypes

import concourse.bass as bass  # noqa: F401  (import keeps bass registered)
from concourse import bacc
import concourse.mybir as mybir
from concourse.tile import TileContext
from concourse.bass_utils import run_bass_kernel_spmd

FP32 = mybir.dt.float32
BF16 = mybir.dt.bfloat16
FP16 = mybir.dt.float16
FP8 = mybir.dt.float8e4

BF = ml_dtypes.bfloat16

NCORES = 8
TOK = 8192          # 4 * 2048 tokens
TPC = TOK // NCORES  # 1024 tokens per core
DIN = 4096
DOUT = 4096
RANK = 16
SCALING = 32.0 / 16

NI = DIN // 128      # 32 contraction blocks
NOC = DOUT // 128    # 32 output chunks of 128
NXQ = 4              # x DMA split (ib-groups)


def _build_program():
    nc = bacc.Bacc("TRN2", target_bir_lowering=False, debug=False,
                   num_devices=NCORES)

    xt_d = nc.dram_tensor("xt", [128, NI, TPC], BF16, kind="ExternalInput")
    wt_d = nc.dram_tensor("wt", [NOC, 128, NI, 128], BF16, kind="ExternalInput")
    wh_d = nc.dram_tensor("wh", [128, NOC, DIN], FP8, kind="ExternalInput")
    at_d = nc.dram_tensor("at", [128, NI, RANK], BF16, kind="ExternalInput")
    a2t_d = nc.dram_tensor("a2t", [128, NI, RANK], BF16, kind="ExternalInput")
    b2t_d = nc.dram_tensor("b2t", [RANK, DOUT], BF16, kind="ExternalInput")
    b2n_d = nc.dram_tensor("b2n", [128, NOC, RANK], BF16, kind="ExternalInput")
    mag_d = nc.dram_tensor("mag", [128, NOC], FP32, kind="ExternalInput")
    out_d = nc.dram_tensor("out", [DOUT, TPC], BF16, kind="ExternalOutput")

    with TileContext(nc) as tc:
        with (
            tc.tile_pool(name="const", bufs=1) as const,
            tc.tile_pool(name="xtp", bufs=1) as xtp,
            tc.tile_pool(name="wtp", bufs=4) as wtp,
            tc.tile_pool(name="whp", bufs=3) as whp,
            tc.tile_pool(name="outp", bufs=6) as outp,
            tc.tile_pool(name="scl", bufs=4) as scl,
            tc.tile_pool(name="mp", bufs=4, space="PSUM") as mp,
            tc.tile_pool(name="np2", bufs=2, space="PSUM") as np2,
            tc.tile_pool(name="xap", bufs=2, space="PSUM") as xap,
        ):
            aT = const.tile([128, NI, RANK], BF16)
            nc.sync.dma_start(aT[:], at_d[:])
            a2T = const.tile([128, NI, RANK], BF16)
            nc.sync.dma_start(a2T[:], a2t_d[:])

            # resident x^T [i_part, i_blk, tok]: one DMA per i-block — a
            # single DMA queue moves only ~23 GB/s, so landing time is set
            # by how many queues a tensor is spread across
            QI = NI // NXQ
            xTq = [xtp.tile([128, QI, TPC], BF16, name=f"xTq{q}")
                   for q in range(NXQ)]
            b2t = const.tile([RANK, DOUT], BF16)
            b2n = const.tile([128, NOC, RANK], BF16)
            mag = const.tile([128, NOC], FP32)

            def xT(ib):
                return xTq[ib // QI][:, ib % QI, :]

            def dma_xq(q):
                for j in range(QI):
                    nc.sync.dma_start(xTq[q][:, j, :], xt_d[:, q * QI + j, :])

            dma_xq(0)

            def chunk_tiles(c, with_wh=True):
                wt_c = wtp.tile([128, NI, 128], BF16, tag="w", name=f"w{c}")
                for h in range(4):
                    nc.sync.dma_start(wt_c[:, h * 8:(h + 1) * 8, :],
                                      wt_d[c, :, h * 8:(h + 1) * 8, :])
                t = {"c": c, "wt": wt_c}
                if with_wh:
                    chunk_wh(t)
                return t

            def chunk_wh(t):
                c = t["c"]
                # n1 = rowsum(W^2) on the scalar engine (fp16 natural
                # layout); one DMA piece per ACT op
                wh_c = whp.tile([128, DIN], FP8, tag="wh", name=f"wh{c}")
                n1p = scl.tile([128, 4], FP32, tag="n1p", name=f"n1p{c}")
                for k in range(4):
                    nc.sync.dma_start(wh_c[:, k * 1024:(k + 1) * 1024],
                                      wh_d[:, c, k * 1024:(k + 1) * 1024])
                    nc.scalar.activation(
                        scl.tile([128, 1024], FP32, tag="sqw",
                                 name=f"sqw{c}_{k}")[:],
                        wh_c[:, k * 1024:(k + 1) * 1024],
                        mybir.ActivationFunctionType.Square,
                        accum_out=n1p[:, k:k + 1])
                n1c = scl.tile([128, 1], FP32, tag="n1c", name=f"n1c{c}")
                nc.vector.reduce_sum(n1c[:], n1p[:], axis=mybir.AxisListType.X)
                t["n1c"] = n1c

            def chunk_psums(t):
                c = t["c"]
                t["ps0"] = mp.tile([128, 512], FP32, tag="mp", name=f"ps0_{c}")
                t["ps1"] = mp.tile([128, 512], FP32, tag="mp", name=f"ps1_{c}")
                t["pn2"] = np2.tile([128, RANK], FP32, tag="np2",
                                    name=f"pn2_{c}")

            def chunk_ib(t, ib, with_pn2=True):
                w = t["wt"][:, ib, :]
                nc.tensor.matmul(t["ps0"][:], w, xT(ib)[:, 0:512],
                                 start=(ib == 0), stop=False)
                if with_pn2:
                    nc.tensor.matmul(t["pn2"][:], w, a2T[:, ib, :],
                                     start=(ib == 0), stop=False)
                nc.tensor.matmul(t["ps1"][:], w, xT(ib)[:, 512:1024],
                                 start=(ib == 0), stop=False)

            def pn2_pass(t):
                for ib in range(NI):
                    nc.tensor.matmul(t["pn2"][:], t["wt"][:, ib, :],
                                     a2T[:, ib, :],
                                     start=(ib == 0), stop=False)

            def finish_norm(t):
                c = t["c"]
                pn2 = t["pn2"]
                b2c = b2t[:, c * 128:(c + 1) * 128]
                # + B2@G into the n2 psum: row norm finishes as one fused
                # multiply-accumulate against B2
                nc.tensor.matmul(pn2[:], b2c, g_sb[:], start=False, stop=True)

                # cross + lowrank norm terms: sum_r pn2[o,r] * B2[o,r]
                cr = scl.tile([128, 1], FP32, tag="cr", name=f"cr{c}")
                nc.vector.scalar_tensor_tensor(
                    out=scl.tile([128, RANK], FP32, tag="scr",
                                 name=f"scr{c}")[:],
                    in0=pn2[:], scalar=1.0, in1=b2n[:, c, :],
                    op0=mybir.AluOpType.mult, op1=mybir.AluOpType.mult,
                    accum_out=cr[:])
                nsq = scl.tile([128, 1], FP32, tag="nsq", name=f"nsq{c}")
                nc.vector.tensor_add(nsq[:], cr[:], t["n1c"][:])
                nrm = scl.tile([128, 1], FP32, tag="nrm", name=f"nrm{c}")
                nc.scalar.activation(nrm[:], nsq[:],
                                     mybir.ActivationFunctionType.Sqrt)
                nc.vector.reciprocal(nrm[:], nrm[:])
                sc = scl.tile([128, 1], FP32, tag="sc", name=f"sc{c}")
                nc.vector.tensor_mul(sc[:], nrm[:], mag[:, c:c + 1])
                t["sc"] = sc

            def finish_apply(t, out_pieces=1):
                c = t["c"]
                b2c = b2t[:, c * 128:(c + 1) * 128]
                # rank-16 DoRA term folded into the out accumulation
                nc.tensor.matmul(t["ps0"][:], b2c, xaT[:, 0:512],
                                 start=False, stop=True)
                nc.tensor.matmul(t["ps1"][:], b2c, xaT[:, 512:1024],
                                 start=False, stop=True)
                for half, ps in ((0, t["ps0"]), (1, t["ps1"])):
                    o_t = outp.tile([128, 512], BF16, tag="o",
                                    name=f"o{half}_{c}")
                    nc.vector.tensor_scalar_mul(o_t[:], ps[:], t["sc"][:])
                    w_p = 512 // out_pieces
                    for h in range(out_pieces):
                        nc.sync.dma_start(
                            out_d[c * 128:(c + 1) * 128,
                                  half * 512 + h * w_p:
                                  half * 512 + (h + 1) * w_p],
                            o_t[:, h * w_p:(h + 1) * w_p])

            # ---- chunks 0+1, interleaved per x i-block group to race the
            # x DMA; xa accumulates alongside ----
            t0 = chunk_tiles(0, with_wh=False)
            t1 = chunk_tiles(1, with_wh=False)
            chunk_psums(t0)
            chunk_psums(t1)

            # G = A @ A^T  [rank, rank] — needs only aT, runs immediately
            # (borrows an np2 bank; drained well before pn2_1 needs it)
            ps_g = np2.tile([RANK, RANK], FP32, tag="np2", name="psg")
            for ib in range(NI):
                nc.tensor.matmul(ps_g[:], aT[:, ib, :], aT[:, ib, :],
                                 start=(ib == 0), stop=(ib == NI - 1))
            g_sb = const.tile([RANK, RANK], BF16)
            nc.vector.tensor_copy(g_sb[:], ps_g[:])

            # xa^T = (x @ A^T)^T [rank, tok]
            xaT = const.tile([RANK, TPC], BF16)
            ps_xa = [xap.tile([RANK, 512], FP32, tag="xap", name=f"psxa{q}")
                     for q in range(2)]

            for q in range(NXQ):
                if q + 1 < NXQ:
                    dma_xq(q + 1)
                if q == 1:
                    chunk_wh(t0)
                    nc.sync.dma_start(b2t[:], b2t_d[:])
                if q == 2:
                    chunk_wh(t1)
                    nc.sync.dma_start(b2n[:], b2n_d[:])
                    nc.sync.dma_start(mag[:], mag_d[:])
                for j in range(QI):
                    ib = q * QI + j
                    nc.tensor.matmul(ps_xa[0][:], aT[:, ib, :],
                                     xT(ib)[:, 0:512],
                                     start=(ib == 0), stop=(ib == NI - 1))
                    nc.tensor.matmul(ps_xa[1][:], aT[:, ib, :],
                                     xT(ib)[:, 512:1024],
                                     start=(ib == 0), stop=(ib == NI - 1))
                    chunk_ib(t0, ib)
                    chunk_ib(t1, ib)
            for q in range(2):
                nc.vector.tensor_copy(xaT[:, q * 512:(q + 1) * 512],
                                      ps_xa[q][:])
            finish_norm(t0)
            finish_apply(t0)
            finish_norm(t1)
            finish_apply(t1)

            # ---- steady-state chunks ----
            for c in range(2, NOC - 1):
                t = chunk_tiles(c)
                chunk_psums(t)
                for ib in range(NI):
                    chunk_ib(t, ib)
                finish_norm(t)
                finish_apply(t)

            # last chunk: its wt is fully prefetched, so run the pn2 pass +
            # norm chain first — the tail then ends right after the folds
            t = chunk_tiles(NOC - 1)
            chunk_psums(t)
            pn2_pass(t)
            finish_norm(t)
            for ib in range(NI):
                chunk_ib(t, ib, with_pn2=False)
            finish_apply(t, out_pieces=4)

    nc.compile()
    return nc


_PROGRAM = None


def _get_program():
    global _PROGRAM
    if _PROGRAM is None:
        _PROGRAM = _build_program()
    return _PROGRAM


def _prep_inputs(x, weight, lora_a_w, lora_b_w, magnitude):
    w32 = weight.astype(np.float32)
    wb = w32.astype(BF)
    wt = np.ascontiguousarray(
        wb.T.reshape(NI, 128, NOC, 128).transpose(2, 1, 0, 3))
    wh = np.ascontiguousarray(
        w32.astype(ml_dtypes.float8_e4m3).reshape(NOC, 128, DIN).transpose(1, 0, 2))
    a32 = lora_a_w.astype(np.float32)
    at = np.ascontiguousarray(
        a32.astype(BF).T.reshape(NI, 128, RANK).transpose(1, 0, 2))
    a2t = np.ascontiguousarray(
        (2.0 * a32).astype(BF).T.reshape(NI, 128, RANK).transpose(1, 0, 2))
    b2 = (SCALING * lora_b_w.astype(np.float32)).astype(BF)
    b2t = np.ascontiguousarray(b2.T)
    b2n = np.ascontiguousarray(
        b2.reshape(NOC, 128, RANK).transpose(1, 0, 2))
    magr = np.ascontiguousarray(
        magnitude.astype(np.float32).reshape(NOC, 128).T)

    xb = x.reshape(TOK, DIN).astype(BF)
    in_maps = []
    for cpu in range(NCORES):
        xs = xb[cpu * TPC:(cpu + 1) * TPC].T
        xt = np.ascontiguousarray(
            xs.reshape(NI, 128, TPC).transpose(1, 0, 2))
        in_maps.append({"xt": xt, "wt": wt, "wh": wh, "at": at, "a2t": a2t,
                        "b2t": b2t, "b2n": b2n, "mag": magr})
    return in_maps


def kernel(x, weight, lora_a_w, lora_b_w, magnitude, _trace=False, **_kw):
    nc = _get_program()
    in_maps = _prep_inputs(x, weight, lora_a_w, lora_b_w, magnitude)
    res = run_bass_kernel_spmd(nc, in_maps, list(range(NCORES)), trace=_trace)
    out = np.empty((TOK, DOUT), dtype=np.float32)
    for c in range(NCORES):
        out[c * TPC:(c + 1) * TPC] = res.results[c]["out"].T.astype(np.float32)
    if _trace:
        kernel._last_results = res
    return out.reshape(4, 2048, DOUT)


# revision 20
# speedup vs baseline: 1.0312x; 1.0012x over previous
"""DoRA linear layer on 8 TRN2 NeuronCores (bf16 tensor-engine path).

out = (magnitude / ||W + s*B@A||_row) * (x @ (W + s*B@A)^T),  s = alpha/rank = 2.

Identity used: the reference's
    dora_out + base_out = mag_norm_scale * (base_out + s * lora_out)
                        = scale_o * (x @ W^T + s * (x @ A^T) @ B^T)

Structure (per core, data-parallel over tokens, 1024 tok/core):
  - stationary = W^T chunk [128i, 128o], moving = x^T [128i, 512t] (bf16:
    1 col/cycle vs ~2 for fp32r on real HW) -> psum out^T tiles [128o, 512t].
  - n2 = 2*(W @ A^T) rides the same stationary as extra 16-col matmuls into
    a [128o, 16] psum; B2@G accumulates into the same psum, so the row norm
    finishes as ONE fused multiply-accumulate against B2 (natural layout)
    plus n1 = rowsum(W^2) from an fp16 W copy on the scalar engine.
  - All norm/scale math lives in o-partition space: scale is a [128,1]
    per-partition broadcast, no transposes, no DRAM round-trip.
  - out^T written bf16; host transposes/casts back to [tok, out] fp32.
"""

import sys

sys.path.insert(0, "/opt/trn_rl_repo")

import numpy as np
import ml_dtypes

import concourse.bass as bass  # noqa: F401  (import keeps bass registered)
from concourse import bacc
import concourse.mybir as mybir
from concourse.tile import TileContext
from concourse.bass_utils import run_bass_kernel_spmd

FP32 = mybir.dt.float32
BF16 = mybir.dt.bfloat16
FP16 = mybir.dt.float16
FP8 = mybir.dt.float8e4

BF = ml_dtypes.bfloat16

NCORES = 8
TOK = 8192          # 4 * 2048 tokens
TPC = TOK // NCORES  # 1024 tokens per core
DIN = 4096
DOUT = 4096
RANK = 16
SCALING = 32.0 / 16

NI = DIN // 128      # 32 contraction blocks
NOC = DOUT // 128    # 32 output chunks of 128
NXQ = 4              # x DMA split (ib-groups)


def _build_program():
    nc = bacc.Bacc("TRN2", target_bir_lowering=False, debug=False,
                   num_devices=NCORES)

    xt_d = nc.dram_tensor("xt", [128, NI, TPC], BF16, kind="ExternalInput")
    wt_d = nc.dram_tensor("wt", [NOC, 128, NI, 128], BF16, kind="ExternalInput")
    wh_d = nc.dram_tensor("wh", [128, NOC, DIN], FP8, kind="ExternalInput")
    at_d = nc.dram_tensor("at", [128, NI, RANK], BF16, kind="ExternalInput")
    a2t_d = nc.dram_tensor("a2t", [128, NI, RANK], BF16, kind="ExternalInput")
    b2t_d = nc.dram_tensor("b2t", [RANK, DOUT], BF16, kind="ExternalInput")
    b2n_d = nc.dram_tensor("b2n", [128, NOC, RANK], BF16, kind="ExternalInput")
    mag_d = nc.dram_tensor("mag", [128, NOC], FP32, kind="ExternalInput")
    out_d = nc.dram_tensor("out", [DOUT, TPC], BF16, kind="ExternalOutput")

    with TileContext(nc) as tc:
        with (
            tc.tile_pool(name="const", bufs=1) as const,
            tc.tile_pool(name="xtp", bufs=1) as xtp,
            tc.tile_pool(name="wtp", bufs=4) as wtp,
            tc.tile_pool(name="whp", bufs=3) as whp,
            tc.tile_pool(name="outp", bufs=6) as outp,
            tc.tile_pool(name="scl", bufs=4) as scl,
            tc.tile_pool(name="mp", bufs=4, space="PSUM") as mp,
            tc.tile_pool(name="np2", bufs=2, space="PSUM") as np2,
            tc.tile_pool(name="xap", bufs=2, space="PSUM") as xap,
        ):
            aT = const.tile([128, NI, RANK], BF16)
            nc.sync.dma_start(aT[:], at_d[:])
            a2T = const.tile([128, NI, RANK], BF16)
            nc.sync.dma_start(a2T[:], a2t_d[:])

            # resident x^T [i_part, i_blk, tok]: one DMA per i-block — a
            # single DMA queue moves only ~23 GB/s, so landing time is set
            # by how many queues a tensor is spread across
            QI = NI // NXQ
            xTq = [xtp.tile([128, QI, TPC], BF16, name=f"xTq{q}")
                   for q in range(NXQ)]
            b2t = const.tile([RANK, DOUT], BF16)
            b2n = const.tile([128, NOC, RANK], BF16)
            mag = const.tile([128, NOC], FP32)

            def xT(ib):
                return xTq[ib // QI][:, ib % QI, :]

            def dma_xq(q):
                for j in range(QI):
                    nc.sync.dma_start(xTq[q][:, j, :], xt_d[:, q * QI + j, :])

            dma_xq(0)

            def chunk_tiles(c, with_wh=True):
                wt_c = wtp.tile([128, NI, 128], BF16, tag="w", name=f"w{c}")
                for h in range(4):
                    nc.sync.dma_start(wt_c[:, h * 8:(h + 1) * 8, :],
                                      wt_d[c, :, h * 8:(h + 1) * 8, :])
                t = {"c": c, "wt": wt_c}
                if with_wh:
                    chunk_wh(t)
                return t

            def chunk_wh(t):
                c = t["c"]
                # n1 = rowsum(W^2) on the scalar engine (fp16 natural
                # layout); one DMA piece per ACT op
                wh_c = whp.tile([128, DIN], FP8, tag="wh", name=f"wh{c}")
                n1p = scl.tile([128, 4], FP32, tag="n1p", name=f"n1p{c}")
                for k in range(4):
                    nc.sync.dma_start(wh_c[:, k * 1024:(k + 1) * 1024],
                                      wh_d[:, c, k * 1024:(k + 1) * 1024])
                    nc.scalar.activation(
                        scl.tile([128, 1024], FP32, tag="sqw",
                                 name=f"sqw{c}_{k}")[:],
                        wh_c[:, k * 1024:(k + 1) * 1024],
                        mybir.ActivationFunctionType.Square,
                        accum_out=n1p[:, k:k + 1])
                n1c = scl.tile([128, 1], FP32, tag="n1c", name=f"n1c{c}")
                nc.vector.reduce_sum(n1c[:], n1p[:], axis=mybir.AxisListType.X)
                t["n1c"] = n1c

            def chunk_psums(t):
                c = t["c"]
                t["ps0"] = mp.tile([128, 512], FP32, tag="mp", name=f"ps0_{c}")
                t["ps1"] = mp.tile([128, 512], FP32, tag="mp", name=f"ps1_{c}")
                t["pn2"] = np2.tile([128, RANK], FP32, tag="np2",
                                    name=f"pn2_{c}")

            def chunk_ib(t, ib, with_pn2=True):
                w = t["wt"][:, ib, :]
                nc.tensor.matmul(t["ps0"][:], w, xT(ib)[:, 0:512],
                                 start=(ib == 0), stop=False)
                if with_pn2:
                    nc.tensor.matmul(t["pn2"][:], w, a2T[:, ib, :],
                                     start=(ib == 0), stop=False)
                nc.tensor.matmul(t["ps1"][:], w, xT(ib)[:, 512:1024],
                                 start=(ib == 0), stop=False)

            def pn2_pass(t):
                for ib in range(NI):
                    nc.tensor.matmul(t["pn2"][:], t["wt"][:, ib, :],
                                     a2T[:, ib, :],
                                     start=(ib == 0), stop=False)

            def finish_norm(t):
                c = t["c"]
                pn2 = t["pn2"]
                b2c = b2t[:, c * 128:(c + 1) * 128]
                # + B2@G into the n2 psum: row norm finishes as one fused
                # multiply-accumulate against B2
                nc.tensor.matmul(pn2[:], b2c, g_sb[:], start=False, stop=True)

                # cross + lowrank norm terms: sum_r pn2[o,r] * B2[o,r]
                cr = scl.tile([128, 1], FP32, tag="cr", name=f"cr{c}")
                nc.vector.scalar_tensor_tensor(
                    out=scl.tile([128, RANK], FP32, tag="scr",
                                 name=f"scr{c}")[:],
                    in0=pn2[:], scalar=1.0, in1=b2n[:, c, :],
                    op0=mybir.AluOpType.mult, op1=mybir.AluOpType.mult,
                    accum_out=cr[:])
                nsq = scl.tile([128, 1], FP32, tag="nsq", name=f"nsq{c}")
                nc.vector.tensor_add(nsq[:], cr[:], t["n1c"][:])
                nrm = scl.tile([128, 1], FP32, tag="nrm", name=f"nrm{c}")
                nc.scalar.activation(nrm[:], nsq[:],
                                     mybir.ActivationFunctionType.Sqrt)
                nc.vector.reciprocal(nrm[:], nrm[:])
                sc = scl.tile([128, 1], FP32, tag="sc", name=f"sc{c}")
                nc.vector.tensor_mul(sc[:], nrm[:], mag[:, c:c + 1])
                t["sc"] = sc

            def finish_apply(t, out_pieces=1):
                c = t["c"]
                b2c = b2t[:, c * 128:(c + 1) * 128]
                # rank-16 DoRA term folded into the out accumulation
                nc.tensor.matmul(t["ps0"][:], b2c, xaT[:, 0:512],
                                 start=False, stop=True)
                nc.tensor.matmul(t["ps1"][:], b2c, xaT[:, 512:1024],
                                 start=False, stop=True)
                for half, ps in ((0, t["ps0"]), (1, t["ps1"])):
                    o_t = outp.tile([128, 512], BF16, tag="o",
                                    name=f"o{half}_{c}")
                    nc.vector.tensor_scalar_mul(o_t[:], ps[:], t["sc"][:])
                    w_p = 512 // out_pieces
                    for h in range(out_pieces):
                        nc.sync.dma_start(
                            out_d[c * 128:(c + 1) * 128,
                                  half * 512 + h * w_p:
                                  half * 512 + (h + 1) * w_p],
                            o_t[:, h * w_p:(h + 1) * w_p])

            # ---- chunks 0+1, interleaved per x i-block group to race the
            # x DMA; xa accumulates alongside ----
            t0 = chunk_tiles(0, with_wh=False)
            t1 = chunk_tiles(1, with_wh=False)
            chunk_psums(t0)
            chunk_psums(t1)

            # G = A @ A^T  [rank, rank] — needs only aT, runs immediately
            # (borrows an np2 bank; drained well before pn2_1 needs it)
            ps_g = np2.tile([RANK, RANK], FP32, tag="np2", name="psg")
            for ib in range(NI):
                nc.tensor.matmul(ps_g[:], aT[:, ib, :], aT[:, ib, :],
                                 start=(ib == 0), stop=(ib == NI - 1))
            g_sb = const.tile([RANK, RANK], BF16)
            nc.vector.tensor_copy(g_sb[:], ps_g[:])

            # xa^T = (x @ A^T)^T [rank, tok]
            xaT = const.tile([RANK, TPC], BF16)
            ps_xa = [xap.tile([RANK, 512], FP32, tag="xap", name=f"psxa{q}")
                     for q in range(2)]

            for q in range(NXQ):
                if q + 1 < NXQ:
                    dma_xq(q + 1)
                if q == 1:
                    chunk_wh(t0)
                    nc.sync.dma_start(b2t[:], b2t_d[:])
                if q == 2:
                    chunk_wh(t1)
                    nc.sync.dma_start(b2n[:], b2n_d[:])
                    nc.sync.dma_start(mag[:], mag_d[:])
                for j in range(QI):
                    ib = q * QI + j
                    nc.tensor.matmul(ps_xa[0][:], aT[:, ib, :],
                                     xT(ib)[:, 0:512],
                                     start=(ib == 0), stop=(ib == NI - 1))
                    nc.tensor.matmul(ps_xa[1][:], aT[:, ib, :],
                                     xT(ib)[:, 512:1024],
                                     start=(ib == 0), stop=(ib == NI - 1))
                    chunk_ib(t0, ib)
                    chunk_ib(t1, ib)
            for q in range(2):
                nc.vector.tensor_copy(xaT[:, q * 512:(q + 1) * 512],
                                      ps_xa[q][:])
            finish_norm(t0)
            finish_apply(t0)
            finish_norm(t1)
            finish_apply(t1)

            # ---- steady-state chunks ----
            for c in range(2, NOC):
                t = chunk_tiles(c)
                chunk_psums(t)
                for ib in range(NI):
                    chunk_ib(t, ib)
                finish_norm(t)
                finish_apply(t, out_pieces=4 if c == NOC - 1 else 1)

    nc.compile()
    return nc


_PROGRAM = None


def _get_program():
    global _PROGRAM
    if _PROGRAM is None:
        _PROGRAM = _build_program()
    return _PROGRAM


def _prep_inputs(x, weight, lora_a_w, lora_b_w, magnitude):
    w32 = weight.astype(np.float32)
    wb = w32.astype(BF)
    wt = np.ascontiguousarray(
        wb.T.reshape(NI, 128, NOC, 128).transpose(2, 1, 0, 3))
    wh = np.ascontiguousarray(
        w32.astype(ml_dtypes.float8_e4m3).reshape(NOC, 128, DIN).transpose(1, 0, 2))
    a32 = lora_a_w.astype(np.float32)
    at = np.ascontiguousarray(
        a32.astype(BF).T.reshape(NI, 128, RANK).transpose(1, 0, 2))
    a2t = np.ascontiguousarray(
        (2.0 * a32).astype(BF).T.reshape(NI, 128, RANK).transpose(1, 0, 2))
    b2 = (SCALING * lora_b_w.astype(np.float32)).astype(BF)
    b2t = np.ascontiguousarray(b2.T)
    b2n = np.ascontiguousarray(
        b2.reshape(NOC, 128, RANK).transpose(1, 0, 2))
    magr = np.ascontiguousarray(
        magnitude.astype(np.float32).reshape(NOC, 128).T)

    xb = x.reshape(TOK, DIN).astype(BF)
    in_maps = []
    for cpu in range(NCORES):
        xs = xb[cpu * TPC:(cpu + 1) * TPC].T
        xt = np.ascontiguousarray(
            xs.reshape(NI, 128, TPC).transpose(1, 0, 2))
        in_maps.append({"xt": xt, "wt": wt, "wh": wh, "at": at, "a2t": a2t,
                        "b2t": b2t, "b2n": b2n, "mag": magr})
    return in_maps


def kernel(x, weight, lora_a_w, lora_b_w, magnitude, _trace=False, **_kw):
    nc = _get_program()
    in_maps = _prep_inputs(x, weight, lora_a_w, lora_b_w, magnitude)
    res = run_bass_kernel_spmd(nc, in_maps, list(range(NCORES)), trace=_trace)
    out = np.empty((TOK, DOUT), dtype=np.float32)
    for c in range(NCORES):
        out[c * TPC:(c + 1) * TPC] = res.results[c]["out"].T.astype(np.float32)
    if _trace:
        kernel._last_results = res
    return out.reshape(4, 2048, DOUT)


# revision 21
# speedup vs baseline: 1.0314x; 1.0002x over previous
"""DoRA linear layer on 8 TRN2 NeuronCores (bf16 tensor-engine path).

out = (magnitude / ||W + s*B@A||_row) * (x @ (W + s*B@A)^T),  s = alpha/rank = 2.

Identity used: the reference's
    dora_out + base_out = mag_norm_scale * (base_out + s * lora_out)
                        = scale_o * (x @ W^T + s * (x @ A^T) @ B^T)

Structure (per core, data-parallel over tokens, 1024 tok/core):
  - stationary = W^T chunk [128i, 128o], moving = x^T [128i, 512t] (bf16:
    1 col/cycle vs ~2 for fp32r on real HW) -> psum out^T tiles [128o, 512t].
  - n2 = 2*(W @ A^T) rides the same stationary as extra 16-col matmuls into
    a [128o, 16] psum; B2@G accumulates into the same psum, so the row norm
    finishes as ONE fused multiply-accumulate against B2 (natural layout)
    plus n1 = rowsum(W^2) from an fp16 W copy on the scalar engine.
  - All norm/scale math lives in o-partition space: scale is a [128,1]
    per-partition broadcast, no transposes, no DRAM round-trip.
  - out^T written bf16; host transposes/casts back to [tok, out] fp32.
"""

import sys

sys.path.insert(0, "/opt/trn_rl_repo")

import numpy as np
import ml_dtypes

import concourse.bass as bass  # noqa: F401  (import keeps bass registered)
from concourse import bacc
import concourse.mybir as mybir
from concourse.tile import TileContext
from concourse.bass_utils import run_bass_kernel_spmd

FP32 = mybir.dt.float32
BF16 = mybir.dt.bfloat16
FP16 = mybir.dt.float16
FP8 = mybir.dt.float8e4

BF = ml_dtypes.bfloat16

NCORES = 8
TOK = 8192          # 4 * 2048 tokens
TPC = TOK // NCORES  # 1024 tokens per core
DIN = 4096
DOUT = 4096
RANK = 16
SCALING = 32.0 / 16

NI = DIN // 128      # 32 contraction blocks
NOC = DOUT // 128    # 32 output chunks of 128
NXQ = 4              # x DMA split (ib-groups)


def _build_program():
    nc = bacc.Bacc("TRN2", target_bir_lowering=False, debug=False,
                   num_devices=NCORES)

    xt_d = nc.dram_tensor("xt", [128, NI, TPC], BF16, kind="ExternalInput")
    wt_d = nc.dram_tensor("wt", [NOC, 128, NI, 128], BF16, kind="ExternalInput")
    wh_d = nc.dram_tensor("wh", [128, NOC, DIN], FP8, kind="ExternalInput")
    at_d = nc.dram_tensor("at", [128, NI, RANK], BF16, kind="ExternalInput")
    a2t_d = nc.dram_tensor("a2t", [128, NI, RANK], BF16, kind="ExternalInput")
    b2t_d = nc.dram_tensor("b2t", [RANK, DOUT], BF16, kind="ExternalInput")
    b2n_d = nc.dram_tensor("b2n", [128, NOC, RANK], BF16, kind="ExternalInput")
    mag_d = nc.dram_tensor("mag", [128, NOC], FP32, kind="ExternalInput")
    out_d = nc.dram_tensor("out", [DOUT, TPC], BF16, kind="ExternalOutput")

    with TileContext(nc) as tc:
        with (
            tc.tile_pool(name="const", bufs=1) as const,
            tc.tile_pool(name="xtp", bufs=1) as xtp,
            tc.tile_pool(name="wtp", bufs=4) as wtp,
            tc.tile_pool(name="whp", bufs=3) as whp,
            tc.tile_pool(name="outp", bufs=6) as outp,
            tc.tile_pool(name="scl", bufs=4) as scl,
            tc.tile_pool(name="mp", bufs=4, space="PSUM") as mp,
            tc.tile_pool(name="np2", bufs=2, space="PSUM") as np2,
            tc.tile_pool(name="xap", bufs=2, space="PSUM") as xap,
        ):
            aT = const.tile([128, NI, RANK], BF16)
            nc.sync.dma_start(aT[:], at_d[:])
            a2T = const.tile([128, NI, RANK], BF16)
            nc.sync.dma_start(a2T[:], a2t_d[:])

            # resident x^T [i_part, i_blk, tok]: one DMA per i-block — a
            # single DMA queue moves only ~23 GB/s, so landing time is set
            # by how many queues a tensor is spread across
            QI = NI // NXQ
            xTq = [xtp.tile([128, QI, TPC], BF16, name=f"xTq{q}")
                   for q in range(NXQ)]
            b2t = const.tile([RANK, DOUT], BF16)
            b2n = const.tile([128, NOC, RANK], BF16)
            mag = const.tile([128, NOC], FP32)

            def xT(ib):
                return xTq[ib // QI][:, ib % QI, :]

            def dma_xq(q):
                for j in range(QI):
                    nc.sync.dma_start(xTq[q][:, j, :], xt_d[:, q * QI + j, :])

            dma_xq(0)

            def chunk_tiles(c, with_wh=True):
                wt_c = wtp.tile([128, NI, 128], BF16, tag="w", name=f"w{c}")
                for h in range(4):
                    nc.sync.dma_start(wt_c[:, h * 8:(h + 1) * 8, :],
                                      wt_d[c, :, h * 8:(h + 1) * 8, :])
                t = {"c": c, "wt": wt_c}
                if with_wh:
                    chunk_wh(t)
                return t

            def chunk_wh(t):
                c = t["c"]
                # n1 = rowsum(W^2) on the scalar engine (fp16 natural
                # layout); one DMA piece per ACT op
                wh_c = whp.tile([128, DIN], FP8, tag="wh", name=f"wh{c}")
                n1p = scl.tile([128, 4], FP32, tag="n1p", name=f"n1p{c}")
                for k in range(4):
                    nc.sync.dma_start(wh_c[:, k * 1024:(k + 1) * 1024],
                                      wh_d[:, c, k * 1024:(k + 1) * 1024])
                    nc.scalar.activation(
                        scl.tile([128, 1024], FP32, tag="sqw",
                                 name=f"sqw{c}_{k}")[:],
                        wh_c[:, k * 1024:(k + 1) * 1024],
                        mybir.ActivationFunctionType.Square,
                        accum_out=n1p[:, k:k + 1])
                n1c = scl.tile([128, 1], FP32, tag="n1c", name=f"n1c{c}")
                nc.vector.reduce_sum(n1c[:], n1p[:], axis=mybir.AxisListType.X)
                t["n1c"] = n1c

            def chunk_psums(t):
                c = t["c"]
                t["ps0"] = mp.tile([128, 512], FP32, tag="mp", name=f"ps0_{c}")
                t["ps1"] = mp.tile([128, 512], FP32, tag="mp", name=f"ps1_{c}")
                t["pn2"] = np2.tile([128, RANK], FP32, tag="np2",
                                    name=f"pn2_{c}")

            def chunk_ib(t, ib, with_pn2=True):
                w = t["wt"][:, ib, :]
                nc.tensor.matmul(t["ps0"][:], w, xT(ib)[:, 0:512],
                                 start=(ib == 0), stop=False)
                nc.tensor.matmul(t["pn2"][:], w, a2T[:, ib, :],
                                 start=(ib == 0), stop=False)
                nc.tensor.matmul(t["ps1"][:], w, xT(ib)[:, 512:1024],
                                 start=(ib == 0), stop=False)

            def pn2_pass(t):
                for ib in range(NI):
                    nc.tensor.matmul(t["pn2"][:], t["wt"][:, ib, :],
                                     a2T[:, ib, :],
                                     start=(ib == 0), stop=False)

            def finish_norm(t):
                c = t["c"]
                pn2 = t["pn2"]
                b2c = b2t[:, c * 128:(c + 1) * 128]
                # + B2@G into the n2 psum: row norm finishes as one fused
                # multiply-accumulate against B2
                nc.tensor.matmul(pn2[:], b2c, g_sb[:], start=False, stop=True)

                # cross + lowrank norm terms: sum_r pn2[o,r] * B2[o,r]
                cr = scl.tile([128, 1], FP32, tag="cr", name=f"cr{c}")
                nc.vector.scalar_tensor_tensor(
                    out=scl.tile([128, RANK], FP32, tag="scr",
                                 name=f"scr{c}")[:],
                    in0=pn2[:], scalar=1.0, in1=b2n[:, c, :],
                    op0=mybir.AluOpType.mult, op1=mybir.AluOpType.mult,
                    accum_out=cr[:])
                nsq = scl.tile([128, 1], FP32, tag="nsq", name=f"nsq{c}")
                nc.vector.tensor_add(nsq[:], cr[:], t["n1c"][:])
                nrm = scl.tile([128, 1], FP32, tag="nrm", name=f"nrm{c}")
                nc.scalar.activation(nrm[:], nsq[:],
                                     mybir.ActivationFunctionType.Sqrt)
                nc.vector.reciprocal(nrm[:], nrm[:])
                sc = scl.tile([128, 1], FP32, tag="sc", name=f"sc{c}")
                nc.vector.tensor_mul(sc[:], nrm[:], mag[:, c:c + 1])
                t["sc"] = sc

            def finish_apply(t, out_pieces=1):
                c = t["c"]
                b2c = b2t[:, c * 128:(c + 1) * 128]
                # rank-16 DoRA term folded into the out accumulation
                nc.tensor.matmul(t["ps0"][:], b2c, xaT[:, 0:512],
                                 start=False, stop=True)
                nc.tensor.matmul(t["ps1"][:], b2c, xaT[:, 512:1024],
                                 start=False, stop=True)
                for half, ps in ((0, t["ps0"]), (1, t["ps1"])):
                    o_t = outp.tile([128, 512], BF16, tag="o",
                                    name=f"o{half}_{c}")
                    nc.vector.tensor_scalar_mul(o_t[:], ps[:], t["sc"][:])
                    w_p = 512 // out_pieces
                    for h in range(out_pieces):
                        nc.sync.dma_start(
                            out_d[c * 128:(c + 1) * 128,
                                  half * 512 + h * w_p:
                                  half * 512 + (h + 1) * w_p],
                            o_t[:, h * w_p:(h + 1) * w_p])

            # ---- chunks 0+1, interleaved per x i-block group to race the
            # x DMA; xa accumulates alongside ----
            t0 = chunk_tiles(0, with_wh=False)
            t1 = chunk_tiles(1, with_wh=False)
            chunk_psums(t0)
            chunk_psums(t1)

            # G = A @ A^T  [rank, rank] — needs only aT, runs immediately
            # (borrows an np2 bank; drained well before pn2_1 needs it)
            ps_g = np2.tile([RANK, RANK], FP32, tag="np2", name="psg")
            for ib in range(NI):
                nc.tensor.matmul(ps_g[:], aT[:, ib, :], aT[:, ib, :],
                                 start=(ib == 0), stop=(ib == NI - 1))
            g_sb = const.tile([RANK, RANK], BF16)
            nc.vector.tensor_copy(g_sb[:], ps_g[:])

            # xa^T = (x @ A^T)^T [rank, tok]
            xaT = const.tile([RANK, TPC], BF16)
            ps_xa = [xap.tile([RANK, 512], FP32, tag="xap", name=f"psxa{q}")
                     for q in range(2)]

            for q in range(NXQ):
                if q + 1 < NXQ:
                    dma_xq(q + 1)
                if q == 1:
                    chunk_wh(t0)
                    nc.sync.dma_start(b2t[:], b2t_d[:])
                if q == 2:
                    chunk_wh(t1)
                    nc.sync.dma_start(b2n[:], b2n_d[:])
                    nc.sync.dma_start(mag[:], mag_d[:])
                for j in range(QI):
                    ib = q * QI + j
                    nc.tensor.matmul(ps_xa[0][:], aT[:, ib, :],
                                     xT(ib)[:, 0:512],
                                     start=(ib == 0), stop=(ib == NI - 1))
                    nc.tensor.matmul(ps_xa[1][:], aT[:, ib, :],
                                     xT(ib)[:, 512:1024],
                                     start=(ib == 0), stop=(ib == NI - 1))
                    chunk_ib(t0, ib)
                    chunk_ib(t1, ib)
            for q in range(2):
                nc.vector.tensor_copy(xaT[:, q * 512:(q + 1) * 512],
                                      ps_xa[q][:])
            finish_norm(t0)
            finish_apply(t0)
            finish_norm(t1)
            finish_apply(t1)

            # ---- steady-state chunks ----
            for c in range(2, NOC):
                t = chunk_tiles(c)
                chunk_psums(t)
                for ib in range(NI):
                    chunk_ib(t, ib)
                finish_norm(t)
                finish_apply(t, out_pieces=4 if c == NOC - 1 else 1)

    nc.compile()
    return nc


_PROGRAM = None


def _get_program():
    global _PROGRAM
    if _PROGRAM is None:
        _PROGRAM = _build_program()
    return _PROGRAM


def _prep_inputs(x, weight, lora_a_w, lora_b_w, magnitude):
    w32 = weight.astype(np.float32)
    wb = w32.astype(BF)
    wt = np.ascontiguousarray(
        wb.T.reshape(NI, 128, NOC, 128).transpose(2, 1, 0, 3))
    wh = np.ascontiguousarray(
        w32.astype(ml_dtypes.float8_e4m3).reshape(NOC, 128, DIN).transpose(1, 0, 2))
    a32 = lora_a_w.astype(np.float32)
    at = np.ascontiguousarray(
        a32.astype(BF).T.reshape(NI, 128, RANK).transpose(1, 0, 2))
    a2t = np.ascontiguousarray(
        (2.0 * a32).astype(BF).T.reshape(NI, 128, RANK).transpose(1, 0, 2))
    b2 = (SCALING * lora_b_w.astype(np.float32)).astype(BF)
    b2t = np.ascontiguousarray(b2.T)
    b2n = np.ascontiguousarray(
        b2.reshape(NOC, 128, RANK).transpose(1, 0, 2))
    magr = np.ascontiguousarray(
        magnitude.astype(np.float32).reshape(NOC, 128).T)

    xb = x.reshape(TOK, DIN).astype(BF)
    in_maps = []
    for cpu in range(NCORES):
        xs = xb[cpu * TPC:(cpu + 1) * TPC].T
        xt = np.ascontiguousarray(
            xs.reshape(NI, 128, TPC).transpose(1, 0, 2))
        in_maps.append({"xt": xt, "wt": wt, "wh": wh, "at": at, "a2t": a2t,
                        "b2t": b2t, "b2n": b2n, "mag": magr})
    return in_maps


def kernel(x, weight, lora_a_w, lora_b_w, magnitude, _trace=False, **_kw):
    nc = _get_program()
    in_maps = _prep_inputs(x, weight, lora_a_w, lora_b_w, magnitude)
    res = run_bass_kernel_spmd(nc, in_maps, list(range(NCORES)), trace=_trace)
    out = np.empty((TOK, DOUT), dtype=np.float32)
    for c in range(NCORES):
        out[c * TPC:(c + 1) * TPC] = res.results[c]["out"].T.astype(np.float32)
    if _trace:
        kernel._last_results = res
    return out.reshape(4, 2048, DOUT)


# revision 22
# speedup vs baseline: 1.0324x; 1.0010x over previous
"""DoRA linear layer on 8 TRN2 NeuronCores (bf16 tensor-engine path).

out = (magnitude / ||W + s*B@A||_row) * (x @ (W + s*B@A)^T),  s = alpha/rank = 2.

Identity used: the reference's
    dora_out + base_out = mag_norm_scale * (base_out + s * lora_out)
                        = scale_o * (x @ W^T + s * (x @ A^T) @ B^T)

Structure (per core, data-parallel over tokens, 1024 tok/core):
  - stationary = W^T chunk [128i, 128o], moving = x^T [128i, 512t] (bf16:
    1 col/cycle vs ~2 for fp32r on real HW) -> psum out^T tiles [128o, 512t].
  - n2 = 2*(W @ A^T) rides the same stationary as extra 16-col matmuls into
    a [128o, 16] psum; B2@G accumulates into the same psum, so the row norm
    finishes as ONE fused multiply-accumulate against B2 (natural layout)
    plus n1 = rowsum(W^2) from an fp16 W copy on the scalar engine.
  - All norm/scale math lives in o-partition space: scale is a [128,1]
    per-partition broadcast, no transposes, no DRAM round-trip.
  - out^T written bf16; host transposes/casts back to [tok, out] fp32.
"""

import sys

sys.path.insert(0, "/opt/trn_rl_repo")

import numpy as np
import ml_dtypes

import concourse.bass as bass  # noqa: F401  (import keeps bass registered)
from concourse import bacc
import concourse.mybir as mybir
from concourse.tile import TileContext
from concourse.bass_utils import run_bass_kernel_spmd

FP32 = mybir.dt.float32
BF16 = mybir.dt.bfloat16
FP16 = mybir.dt.float16
FP8 = mybir.dt.float8e4

BF = ml_dtypes.bfloat16

NCORES = 8
TOK = 8192          # 4 * 2048 tokens
TPC = TOK // NCORES  # 1024 tokens per core
DIN = 4096
DOUT = 4096
RANK = 16
SCALING = 32.0 / 16

NI = DIN // 128      # 32 contraction blocks
NOC = DOUT // 128    # 32 output chunks of 128
NXQ = 4              # x DMA split (ib-groups)


def _build_program():
    nc = bacc.Bacc("TRN2", target_bir_lowering=False, debug=False,
                   num_devices=NCORES)

    xt_d = nc.dram_tensor("xt", [128, NI, TPC], BF16, kind="ExternalInput")
    wt_d = nc.dram_tensor("wt", [NOC, 128, NI, 128], BF16, kind="ExternalInput")
    wh_d = nc.dram_tensor("wh", [128, NOC, DIN], FP8, kind="ExternalInput")
    at_d = nc.dram_tensor("at", [128, NI, RANK], BF16, kind="ExternalInput")
    a2t_d = nc.dram_tensor("a2t", [128, NI, RANK], BF16, kind="ExternalInput")
    b2t_d = nc.dram_tensor("b2t", [RANK, DOUT], BF16, kind="ExternalInput")
    b2n_d = nc.dram_tensor("b2n", [128, NOC, RANK], BF16, kind="ExternalInput")
    mag_d = nc.dram_tensor("mag", [128, NOC], FP32, kind="ExternalInput")
    out_d = nc.dram_tensor("out", [DOUT, TPC], BF16, kind="ExternalOutput")

    with TileContext(nc) as tc:
        with (
            tc.tile_pool(name="const", bufs=1) as const,
            tc.tile_pool(name="xtp", bufs=1) as xtp,
            tc.tile_pool(name="wtp", bufs=3) as wtp,
            tc.tile_pool(name="whp", bufs=2) as whp,
            tc.tile_pool(name="outp", bufs=6) as outp,
            tc.tile_pool(name="scl", bufs=4) as scl,
            tc.tile_pool(name="mp", bufs=4, space="PSUM") as mp,
            tc.tile_pool(name="np2", bufs=2, space="PSUM") as np2,
            tc.tile_pool(name="xap", bufs=2, space="PSUM") as xap,
        ):
            aT = const.tile([128, NI, RANK], BF16)
            nc.sync.dma_start(aT[:], at_d[:])
            a2T = const.tile([128, NI, RANK], BF16)
            nc.sync.dma_start(a2T[:], a2t_d[:])

            # resident x^T [i_part, i_blk, tok]: one DMA per i-block — a
            # single DMA queue moves only ~23 GB/s, so landing time is set
            # by how many queues a tensor is spread across
            QI = NI // NXQ
            xTq = [xtp.tile([128, QI, TPC], BF16, name=f"xTq{q}")
                   for q in range(NXQ)]
            b2t = const.tile([RANK, DOUT], BF16)
            b2n = const.tile([128, NOC, RANK], BF16)
            mag = const.tile([128, NOC], FP32)

            def xT(ib):
                return xTq[ib // QI][:, ib % QI, :]

            def dma_xq(q):
                for j in range(QI):
                    nc.sync.dma_start(xTq[q][:, j, :], xt_d[:, q * QI + j, :])

            dma_xq(0)

            def chunk_tiles(c, with_wh=True):
                wt_c = wtp.tile([128, NI, 128], BF16, tag="w", name=f"w{c}")
                for h in range(4):
                    nc.sync.dma_start(wt_c[:, h * 8:(h + 1) * 8, :],
                                      wt_d[c, :, h * 8:(h + 1) * 8, :])
                t = {"c": c, "wt": wt_c}
                if with_wh:
                    chunk_wh(t)
                return t

            def chunk_wh(t):
                c = t["c"]
                # n1 = rowsum(W^2) on the scalar engine (fp16 natural
                # layout); one DMA piece per ACT op
                wh_c = whp.tile([128, DIN], FP8, tag="wh", name=f"wh{c}")
                n1p = scl.tile([128, 4], FP32, tag="n1p", name=f"n1p{c}")
                for k in range(4):
                    nc.sync.dma_start(wh_c[:, k * 1024:(k + 1) * 1024],
                                      wh_d[:, c, k * 1024:(k + 1) * 1024])
                    nc.scalar.activation(
                        scl.tile([128, 1024], FP32, tag="sqw",
                                 name=f"sqw{c}_{k}")[:],
                        wh_c[:, k * 1024:(k + 1) * 1024],
                        mybir.ActivationFunctionType.Square,
                        accum_out=n1p[:, k:k + 1])
                n1c = scl.tile([128, 1], FP32, tag="n1c", name=f"n1c{c}")
                nc.vector.reduce_sum(n1c[:], n1p[:], axis=mybir.AxisListType.X)
                t["n1c"] = n1c

            def chunk_psums(t):
                c = t["c"]
                t["ps0"] = mp.tile([128, 512], FP32, tag="mp", name=f"ps0_{c}")
                t["ps1"] = mp.tile([128, 512], FP32, tag="mp", name=f"ps1_{c}")
                t["pn2"] = np2.tile([128, RANK], FP32, tag="np2",
                                    name=f"pn2_{c}")

            def chunk_ib(t, ib, with_pn2=True):
                w = t["wt"][:, ib, :]
                nc.tensor.matmul(t["ps0"][:], w, xT(ib)[:, 0:512],
                                 start=(ib == 0), stop=False)
                nc.tensor.matmul(t["pn2"][:], w, a2T[:, ib, :],
                                 start=(ib == 0), stop=False)
                nc.tensor.matmul(t["ps1"][:], w, xT(ib)[:, 512:1024],
                                 start=(ib == 0), stop=False)

            def pn2_pass(t):
                for ib in range(NI):
                    nc.tensor.matmul(t["pn2"][:], t["wt"][:, ib, :],
                                     a2T[:, ib, :],
                                     start=(ib == 0), stop=False)

            def finish_norm(t):
                c = t["c"]
                pn2 = t["pn2"]
                b2c = b2t[:, c * 128:(c + 1) * 128]
                # + B2@G into the n2 psum: row norm finishes as one fused
                # multiply-accumulate against B2
                nc.tensor.matmul(pn2[:], b2c, g_sb[:], start=False, stop=True)

                # cross + lowrank norm terms: sum_r pn2[o,r] * B2[o,r]
                cr = scl.tile([128, 1], FP32, tag="cr", name=f"cr{c}")
                nc.vector.scalar_tensor_tensor(
                    out=scl.tile([128, RANK], FP32, tag="scr",
                                 name=f"scr{c}")[:],
                    in0=pn2[:], scalar=1.0, in1=b2n[:, c, :],
                    op0=mybir.AluOpType.mult, op1=mybir.AluOpType.mult,
                    accum_out=cr[:])
                nsq = scl.tile([128, 1], FP32, tag="nsq", name=f"nsq{c}")
                nc.vector.tensor_add(nsq[:], cr[:], t["n1c"][:])
                nrm = scl.tile([128, 1], FP32, tag="nrm", name=f"nrm{c}")
                nc.scalar.activation(nrm[:], nsq[:],
                                     mybir.ActivationFunctionType.Sqrt)
                nc.vector.reciprocal(nrm[:], nrm[:])
                sc = scl.tile([128, 1], FP32, tag="sc", name=f"sc{c}")
                nc.vector.tensor_mul(sc[:], nrm[:], mag[:, c:c + 1])
                t["sc"] = sc

            def finish_apply(t, out_pieces=1):
                c = t["c"]
                b2c = b2t[:, c * 128:(c + 1) * 128]
                # rank-16 DoRA term folded into the out accumulation
                nc.tensor.matmul(t["ps0"][:], b2c, xaT[:, 0:512],
                                 start=False, stop=True)
                nc.tensor.matmul(t["ps1"][:], b2c, xaT[:, 512:1024],
                                 start=False, stop=True)
                for half, ps in ((0, t["ps0"]), (1, t["ps1"])):
                    o_t = outp.tile([128, 512], BF16, tag="o",
                                    name=f"o{half}_{c}")
                    nc.vector.tensor_scalar_mul(o_t[:], ps[:], t["sc"][:])
                    w_p = 512 // out_pieces
                    for h in range(out_pieces):
                        nc.sync.dma_start(
                            out_d[c * 128:(c + 1) * 128,
                                  half * 512 + h * w_p:
                                  half * 512 + (h + 1) * w_p],
                            o_t[:, h * w_p:(h + 1) * w_p])

            # ---- chunks 0+1, interleaved per x i-block group to race the
            # x DMA; xa accumulates alongside ----
            t0 = chunk_tiles(0, with_wh=False)
            t1 = chunk_tiles(1, with_wh=False)
            chunk_psums(t0)
            chunk_psums(t1)

            # G = A @ A^T  [rank, rank] — needs only aT, runs immediately
            # (borrows an np2 bank; drained well before pn2_1 needs it)
            ps_g = np2.tile([RANK, RANK], FP32, tag="np2", name="psg")
            for ib in range(NI):
                nc.tensor.matmul(ps_g[:], aT[:, ib, :], aT[:, ib, :],
                                 start=(ib == 0), stop=(ib == NI - 1))
            g_sb = const.tile([RANK, RANK], BF16)
            nc.vector.tensor_copy(g_sb[:], ps_g[:])

            # xa^T = (x @ A^T)^T [rank, tok]
            xaT = const.tile([RANK, TPC], BF16)
            ps_xa = [xap.tile([RANK, 512], FP32, tag="xap", name=f"psxa{q}")
                     for q in range(2)]

            for q in range(NXQ):
                if q + 1 < NXQ:
                    dma_xq(q + 1)
                if q == 1:
                    chunk_wh(t0)
                    nc.sync.dma_start(b2t[:], b2t_d[:])
                if q == 2:
                    chunk_wh(t1)
                    nc.sync.dma_start(b2n[:], b2n_d[:])
                    nc.sync.dma_start(mag[:], mag_d[:])
                for j in range(QI):
                    ib = q * QI + j
                    nc.tensor.matmul(ps_xa[0][:], aT[:, ib, :],
                                     xT(ib)[:, 0:512],
                                     start=(ib == 0), stop=(ib == NI - 1))
                    nc.tensor.matmul(ps_xa[1][:], aT[:, ib, :],
                                     xT(ib)[:, 512:1024],
                                     start=(ib == 0), stop=(ib == NI - 1))
                    chunk_ib(t0, ib)
                    chunk_ib(t1, ib)
            for q in range(2):
                nc.vector.tensor_copy(xaT[:, q * 512:(q + 1) * 512],
                                      ps_xa[q][:])
            finish_norm(t0)
            finish_apply(t0)
            finish_norm(t1)
            finish_apply(t1)

            # ---- steady-state chunks ----
            for c in range(2, NOC):
                t = chunk_tiles(c)
                chunk_psums(t)
                for ib in range(NI):
                    chunk_ib(t, ib)
                finish_norm(t)
                finish_apply(t, out_pieces=4 if c == NOC - 1 else 1)

    nc.compile()
    return nc


_PROGRAM = None


def _get_program():
    global _PROGRAM
    if _PROGRAM is None:
        _PROGRAM = _build_program()
    return _PROGRAM


def _prep_inputs(x, weight, lora_a_w, lora_b_w, magnitude):
    w32 = weight.astype(np.float32)
    wb = w32.astype(BF)
    wt = np.ascontiguousarray(
        wb.T.reshape(NI, 128, NOC, 128).transpose(2, 1, 0, 3))
    wh = np.ascontiguousarray(
        w32.astype(ml_dtypes.float8_e4m3).reshape(NOC, 128, DIN).transpose(1, 0, 2))
    a32 = lora_a_w.astype(np.float32)
    at = np.ascontiguousarray(
        a32.astype(BF).T.reshape(NI, 128, RANK).transpose(1, 0, 2))
    a2t = np.ascontiguousarray(
        (2.0 * a32).astype(BF).T.reshape(NI, 128, RANK).transpose(1, 0, 2))
    b2 = (SCALING * lora_b_w.astype(np.float32)).astype(BF)
    b2t = np.ascontiguousarray(b2.T)
    b2n = np.ascontiguousarray(
        b2.reshape(NOC, 128, RANK).transpose(1, 0, 2))
    magr = np.ascontiguousarray(
        magnitude.astype(np.float32).reshape(NOC, 128).T)

    xb = x.reshape(TOK, DIN).astype(BF)
    in_maps = []
    for cpu in range(NCORES):
        xs = xb[cpu * TPC:(cpu + 1) * TPC].T
        xt = np.ascontiguousarray(
            xs.reshape(NI, 128, TPC).transpose(1, 0, 2))
        in_maps.append({"xt": xt, "wt": wt, "wh": wh, "at": at, "a2t": a2t,
                        "b2t": b2t, "b2n": b2n, "mag": magr})
    return in_maps


def kernel(x, weight, lora_a_w, lora_b_w, magnitude, _trace=False, **_kw):
    nc = _get_program()
    in_maps = _prep_inputs(x, weight, lora_a_w, lora_b_w, magnitude)
    res = run_bass_kernel_spmd(nc, in_maps, list(range(NCORES)), trace=_trace)
    out = np.empty((TOK, DOUT), dtype=np.float32)
    for c in range(NCORES):
        out[c * TPC:(c + 1) * TPC] = res.results[c]["out"].T.astype(np.float32)
    if _trace:
        kernel._last_results = res
    return out.reshape(4, 2048, DOUT)


# revision 23
# speedup vs baseline: 1.0329x; 1.0005x over previous
"""DoRA linear layer on 8 TRN2 NeuronCores (bf16 tensor-engine path).

out = (magnitude / ||W + s*B@A||_row) * (x @ (W + s*B@A)^T),  s = alpha/rank = 2.

Identity used: the reference's
    dora_out + base_out = mag_norm_scale * (base_out + s * lora_out)
                        = scale_o * (x @ W^T + s * (x @ A^T) @ B^T)

Structure (per core, data-parallel over tokens, 1024 tok/core):
  - stationary = W^T chunk [128i, 128o], moving = x^T [128i, 512t] (bf16:
    1 col/cycle vs ~2 for fp32r on real HW) -> psum out^T tiles [128o, 512t].
  - n2 = 2*(W @ A^T) rides the same stationary as extra 16-col matmuls into
    a [128o, 16] psum; B2@G accumulates into the same psum, so the row norm
    finishes as ONE fused multiply-accumulate against B2 (natural layout)
    plus n1 = rowsum(W^2) from an fp8 W copy on the scalar engine.
  - All norm/scale math lives in o-partition space: scale is a [128,1]
    per-partition broadcast, no transposes, no DRAM round-trip.
  - out^T written bf16; host transposes/casts back to [tok, out] fp32.
"""

import sys

sys.path.insert(0, "/opt/trn_rl_repo")

import numpy as np
import ml_dtypes

import concourse.bass as bass  # noqa: F401  (import keeps bass registered)
from concourse import bacc
import concourse.mybir as mybir
from concourse.tile import TileContext
from concourse.bass_utils import run_bass_kernel_spmd

FP32 = mybir.dt.float32
BF16 = mybir.dt.bfloat16
FP16 = mybir.dt.float16
FP8 = mybir.dt.float8e4

BF = ml_dtypes.bfloat16

NCORES = 8
TOK = 8192          # 4 * 2048 tokens
TPC = TOK // NCORES  # 1024 tokens per core
DIN = 4096
DOUT = 4096
RANK = 16
SCALING = 32.0 / 16

NI = DIN // 128      # 32 contraction blocks
NOC = DOUT // 128    # 32 output chunks of 128
NXQ = 4              # x DMA split (ib-groups)


def _build_program():
    nc = bacc.Bacc("TRN2", target_bir_lowering=False, debug=False,
                   num_devices=NCORES)

    xt_d = nc.dram_tensor("xt", [128, NI, TPC], BF16, kind="ExternalInput")
    wt_d = nc.dram_tensor("wt", [NOC, 128, NI, 128], BF16, kind="ExternalInput")
    wh_d = nc.dram_tensor("wh", [128, NOC, DIN], FP8, kind="ExternalInput")
    at_d = nc.dram_tensor("at", [128, NI, RANK], BF16, kind="ExternalInput")
    a2t_d = nc.dram_tensor("a2t", [128, NI, RANK], BF16, kind="ExternalInput")
    b2t_d = nc.dram_tensor("b2t", [RANK, DOUT], BF16, kind="ExternalInput")
    b2n_d = nc.dram_tensor("b2n", [128, NOC, RANK], BF16, kind="ExternalInput")
    mag_d = nc.dram_tensor("mag", [128, NOC], FP32, kind="ExternalInput")
    out_d = nc.dram_tensor("out", [DOUT, TPC], BF16, kind="ExternalOutput")

    with TileContext(nc) as tc:
        with (
            tc.tile_pool(name="const", bufs=1) as const,
            tc.tile_pool(name="xtp", bufs=1) as xtp,
            tc.tile_pool(name="wtp", bufs=3) as wtp,
            tc.tile_pool(name="whp", bufs=2) as whp,
            tc.tile_pool(name="outp", bufs=6) as outp,
            tc.tile_pool(name="scl", bufs=4) as scl,
            tc.tile_pool(name="mp", bufs=4, space="PSUM") as mp,
            tc.tile_pool(name="np2", bufs=2, space="PSUM") as np2,
            tc.tile_pool(name="xap", bufs=2, space="PSUM") as xap,
        ):
            aT = const.tile([128, NI, RANK], BF16)
            nc.sync.dma_start(aT[:], at_d[:])
            a2T = const.tile([128, NI, RANK], BF16)
            nc.sync.dma_start(a2T[:], a2t_d[:])

            # resident x^T [i_part, i_blk, tok]: one DMA per i-block — a
            # single DMA queue moves only ~23 GB/s, so landing time is set
            # by how many queues a tensor is spread across
            QI = NI // NXQ
            xTq = [xtp.tile([128, QI, TPC], BF16, name=f"xTq{q}")
                   for q in range(NXQ)]
            b2t = const.tile([RANK, DOUT], BF16)
            b2n = const.tile([128, NOC, RANK], BF16)
            mag = const.tile([128, NOC], FP32)

            def xT(ib):
                return xTq[ib // QI][:, ib % QI, :]

            def dma_xq(q):
                for j in range(QI):
                    nc.sync.dma_start(xTq[q][:, j, :], xt_d[:, q * QI + j, :])

            dma_xq(0)

            def chunk_tiles(c, with_wh=True):
                wt_c = wtp.tile([128, NI, 128], BF16, tag="w", name=f"w{c}")
                for h in range(4):
                    nc.sync.dma_start(wt_c[:, h * 8:(h + 1) * 8, :],
                                      wt_d[c, :, h * 8:(h + 1) * 8, :])
                t = {"c": c, "wt": wt_c}
                if with_wh:
                    chunk_wh(t)
                return t

            def chunk_wh(t):
                c = t["c"]
                # n1 = rowsum(W^2) on the scalar engine (fp16 natural
                # layout); one DMA piece per ACT op
                wh_c = whp.tile([128, DIN], FP8, tag="wh", name=f"wh{c}")
                n1p = scl.tile([128, 4], FP32, tag="n1p", name=f"n1p{c}")
                for k in range(4):
                    nc.sync.dma_start(wh_c[:, k * 1024:(k + 1) * 1024],
                                      wh_d[:, c, k * 1024:(k + 1) * 1024])
                    nc.scalar.activation(
                        scl.tile([128, 1024], FP32, tag="sqw",
                                 name=f"sqw{c}_{k}")[:],
                        wh_c[:, k * 1024:(k + 1) * 1024],
                        mybir.ActivationFunctionType.Square,
                        accum_out=n1p[:, k:k + 1])
                n1c = scl.tile([128, 1], FP32, tag="n1c", name=f"n1c{c}")
                nc.vector.reduce_sum(n1c[:], n1p[:], axis=mybir.AxisListType.X)
                t["n1c"] = n1c

            def chunk_psums(t):
                c = t["c"]
                t["ps0"] = mp.tile([128, 512], FP32, tag="mp", name=f"ps0_{c}")
                t["ps1"] = mp.tile([128, 512], FP32, tag="mp", name=f"ps1_{c}")
                t["pn2"] = np2.tile([128, RANK], FP32, tag="np2",
                                    name=f"pn2_{c}")

            def chunk_ib(t, ib):
                w = t["wt"][:, ib, :]
                nc.tensor.matmul(t["ps0"][:], w, xT(ib)[:, 0:512],
                                 start=(ib == 0), stop=False)
                nc.tensor.matmul(t["pn2"][:], w, a2T[:, ib, :],
                                 start=(ib == 0), stop=False)
                nc.tensor.matmul(t["ps1"][:], w, xT(ib)[:, 512:1024],
                                 start=(ib == 0), stop=False)

            def finish_norm(t):
                c = t["c"]
                pn2 = t["pn2"]
                b2c = b2t[:, c * 128:(c + 1) * 128]
                # + B2@G into the n2 psum: row norm finishes as one fused
                # multiply-accumulate against B2
                nc.tensor.matmul(pn2[:], b2c, g_sb[:], start=False, stop=True)

                # cross + lowrank norm terms: sum_r pn2[o,r] * B2[o,r]
                cr = scl.tile([128, 1], FP32, tag="cr", name=f"cr{c}")
                nc.vector.scalar_tensor_tensor(
                    out=scl.tile([128, RANK], FP32, tag="scr",
                                 name=f"scr{c}")[:],
                    in0=pn2[:], scalar=1.0, in1=b2n[:, c, :],
                    op0=mybir.AluOpType.mult, op1=mybir.AluOpType.mult,
                    accum_out=cr[:])
                nsq = scl.tile([128, 1], FP32, tag="nsq", name=f"nsq{c}")
                nc.vector.tensor_add(nsq[:], cr[:], t["n1c"][:])
                nrm = scl.tile([128, 1], FP32, tag="nrm", name=f"nrm{c}")
                nc.scalar.activation(nrm[:], nsq[:],
                                     mybir.ActivationFunctionType.Sqrt)
                nc.vector.reciprocal(nrm[:], nrm[:])
                sc = scl.tile([128, 1], FP32, tag="sc", name=f"sc{c}")
                nc.vector.tensor_mul(sc[:], nrm[:], mag[:, c:c + 1])
                t["sc"] = sc

            def finish_apply(t, out_pieces=1):
                c = t["c"]
                b2c = b2t[:, c * 128:(c + 1) * 128]
                # rank-16 DoRA term folded into the out accumulation
                nc.tensor.matmul(t["ps0"][:], b2c, xaT[:, 0:512],
                                 start=False, stop=True)
                nc.tensor.matmul(t["ps1"][:], b2c, xaT[:, 512:1024],
                                 start=False, stop=True)
                for half, ps in ((0, t["ps0"]), (1, t["ps1"])):
                    o_t = outp.tile([128, 512], BF16, tag="o",
                                    name=f"o{half}_{c}")
                    nc.vector.tensor_scalar_mul(o_t[:], ps[:], t["sc"][:])
                    w_p = 512 // out_pieces
                    for h in range(out_pieces):
                        nc.sync.dma_start(
                            out_d[c * 128:(c + 1) * 128,
                                  half * 512 + h * w_p:
                                  half * 512 + (h + 1) * w_p],
                            o_t[:, h * w_p:(h + 1) * w_p])

            # ---- chunks 0+1, interleaved per x i-block group to race the
            # x DMA; xa accumulates alongside ----
            t0 = chunk_tiles(0, with_wh=False)
            t1 = chunk_tiles(1, with_wh=False)
            chunk_psums(t0)
            chunk_psums(t1)

            # G = A @ A^T  [rank, rank] — needs only aT, runs immediately
            # (borrows an np2 bank; drained well before pn2_1 needs it)
            ps_g = np2.tile([RANK, RANK], FP32, tag="np2", name="psg")
            for ib in range(NI):
                nc.tensor.matmul(ps_g[:], aT[:, ib, :], aT[:, ib, :],
                                 start=(ib == 0), stop=(ib == NI - 1))
            g_sb = const.tile([RANK, RANK], BF16)
            nc.vector.tensor_copy(g_sb[:], ps_g[:])

            # xa^T = (x @ A^T)^T [rank, tok]
            xaT = const.tile([RANK, TPC], BF16)
            ps_xa = [xap.tile([RANK, 512], FP32, tag="xap", name=f"psxa{q}")
                     for q in range(2)]

            for q in range(NXQ):
                if q + 1 < NXQ:
                    dma_xq(q + 1)
                if q == 1:
                    chunk_wh(t0)
                    nc.sync.dma_start(b2t[:], b2t_d[:])
                if q == 2:
                    chunk_wh(t1)
                    nc.sync.dma_start(b2n[:], b2n_d[:])
                    nc.sync.dma_start(mag[:], mag_d[:])
                for j in range(QI):
                    ib = q * QI + j
                    nc.tensor.matmul(ps_xa[0][:], aT[:, ib, :],
                                     xT(ib)[:, 0:512],
                                     start=(ib == 0), stop=(ib == NI - 1))
                    nc.tensor.matmul(ps_xa[1][:], aT[:, ib, :],
                                     xT(ib)[:, 512:1024],
                                     start=(ib == 0), stop=(ib == NI - 1))
                    chunk_ib(t0, ib)
                    chunk_ib(t1, ib)
            for q in range(2):
                nc.vector.tensor_copy(xaT[:, q * 512:(q + 1) * 512],
                                      ps_xa[q][:])
            finish_norm(t0)
            finish_apply(t0)
            finish_norm(t1)
            finish_apply(t1)

            # ---- steady-state chunks ----
            for c in range(2, NOC):
                t = chunk_tiles(c)
                chunk_psums(t)
                for ib in range(NI):
                    chunk_ib(t, ib)
                finish_norm(t)
                finish_apply(t, out_pieces=4 if c == NOC - 1 else 1)

    nc.compile()
    return nc


_PROGRAM = None


def _get_program():
    global _PROGRAM
    if _PROGRAM is None:
        _PROGRAM = _build_program()
    return _PROGRAM


def _prep_inputs(x, weight, lora_a_w, lora_b_w, magnitude):
    w32 = weight.astype(np.float32)
    wb = w32.astype(BF)
    wt = np.ascontiguousarray(
        wb.T.reshape(NI, 128, NOC, 128).transpose(2, 1, 0, 3))
    wh = np.ascontiguousarray(
        w32.astype(ml_dtypes.float8_e4m3).reshape(NOC, 128, DIN).transpose(1, 0, 2))
    a32 = lora_a_w.astype(np.float32)
    at = np.ascontiguousarray(
        a32.astype(BF).T.reshape(NI, 128, RANK).transpose(1, 0, 2))
    a2t = np.ascontiguousarray(
        (2.0 * a32).astype(BF).T.reshape(NI, 128, RANK).transpose(1, 0, 2))
    b2 = (SCALING * lora_b_w.astype(np.float32)).astype(BF)
    b2t = np.ascontiguousarray(b2.T)
    b2n = np.ascontiguousarray(
        b2.reshape(NOC, 128, RANK).transpose(1, 0, 2))
    magr = np.ascontiguousarray(
        magnitude.astype(np.float32).reshape(NOC, 128).T)

    xb = x.reshape(TOK, DIN).astype(BF)
    in_maps = []
    for cpu in range(NCORES):
        xs = xb[cpu * TPC:(cpu + 1) * TPC].T
        xt = np.ascontiguousarray(
            xs.reshape(NI, 128, TPC).transpose(1, 0, 2))
        in_maps.append({"xt": xt, "wt": wt, "wh": wh, "at": at, "a2t": a2t,
                        "b2t": b2t, "b2n": b2n, "mag": magr})
    return in_maps


def kernel(x, weight, lora_a_w, lora_b_w, magnitude, _trace=False, **_kw):
    nc = _get_program()
    in_maps = _prep_inputs(x, weight, lora_a_w, lora_b_w, magnitude)
    res = run_bass_kernel_spmd(nc, in_maps, list(range(NCORES)), trace=_trace)
    out = np.empty((TOK, DOUT), dtype=np.float32)
    for c in range(NCORES):
        out[c * TPC:(c + 1) * TPC] = res.results[c]["out"].T.astype(np.float32)
    if _trace:
        kernel._last_results = res
    return out.reshape(4, 2048, DOUT)
